# revision 6
# baseline (speedup 1.0000x reference)
"""Trainium2 Bass kernel for nn_IntegerCifar10Net (quantized VGG-ish CNN).

Data parallel over 8 NeuronCores, B=64 images/core.

v3: engine-balanced quant chain + upfront chunked xcol streaming.

Layer matmul schemes (unchanged from v2 except L6):
  L1 : exact 3-plane bf16 im2col (K=81), co=64; x-halves to PSUM partitions
       0-63 / 64-127; quantized row writes A2 lower+upper halves in one op.
  L2 : "halves" trick - PSUM partitions 0-63 = left 16 output cols,
       64-127 = right 16; block-diagonal weights, 5 DR matmuls per bank.
  L3 : dx-packing (x+1 dup in upper partitions): 3 DR matmuls per bank.
  L4/L5: plain 9 taps -> 5 DR matmuls per bank.
  L6 : 9 DR matmuls per bank - the two odd 9th taps of the two ci-groups
       share one DR pair (cig-pairing) instead of 2 zero-padded pairs.

Quant chain per bank, split across three engines so no engine gates PE:
  Scalar (Act) : z = relu(psum*scale + bias)        [PSUM -> SBUF f32]
  Vector (DVE) : u = min(z + MAGIC, MAGIC+7)        [RNE round + clamp]
  GpSimd(Pool) : a = u - MAGIC -> fp8               [A-tensor write]
Max-pool layers additionally run the pairwise maxes on DVE (Pool has no
native tensor_tensor on TRN2) over pre-round z; round commutes with max.

xcol is staged fully in SBUF via 8 upfront chunk DMAs (4 rows each)
round-robined over the sync/scalar/gpsimd queues; weights stream behind.
"""

import sys
import numpy as np

sys.path.insert(0, "/opt/trn_rl_repo")

import ml_dtypes

N_CORES = 8
B = 64  # images per core
MAGIC = 12582912.0  # 1.5 * 2^23 : RNE rounding magic for |v| < 2^22
N_PLANES = 3  # bf16 planes for exact L1 input (hi/mid/lo)
K1 = 27 * N_PLANES

# tap pair schedule for 3x3 convs: pairs of taps t=(dy,dx) row-major,
# 10th tap is zero-weight padding with moving delta -B (always in bounds)
TAPS = [(dy, dx) for dy in range(3) for dx in range(3)]


def _pair_deltas(W):
    """Moving-AP element deltas between the two taps of each DR pair."""
    ds = []
    for p in range(4):
        (dya, dxa), (dyb, dxb) = TAPS[2 * p], TAPS[2 * p + 1]
        ds.append(((dyb - dya) * W + (dxb - dxa)) * B)
    ds.append(-B)  # pad pair: (t8, zero-weight tap at x-1)
    return ds


# ----------------------------------------------------------------------------
# Host-side packing
# ----------------------------------------------------------------------------

def _qint(w):
    """round(clip(w,-1,1)*7) as float32 integers, matching jax fp32 chain."""
    w = np.asarray(w, np.float32)
    return np.round(np.clip(w, -1.0, 1.0) * np.float32(7.0)).astype(np.float32)


def _scale_bias(g, b, denom):
    # z7 = conv_int * (7*g/denom) + 7*b, constants in f64 then rounded to f32
    s = (7.0 * np.asarray(g, np.float64) / denom).astype(np.float32)
    bt = (7.0 * np.asarray(b, np.float64)).astype(np.float32)
    return np.ascontiguousarray(np.stack([s, bt], axis=1))  # [co, 2] f32


def _im2col_bf16(x):
    """x [B,3,32,32] f32 -> [K1, 32, 32, B] bf16 (N_PLANES x 27 rows)."""
    Bc = x.shape[0]
    xp = np.zeros((Bc, 3, 34, 34), np.float32)
    xp[:, :, 1:33, 1:33] = x
    planes = np.empty((27, 32, 32, Bc), np.float32)
    k = 0
    for ci in range(3):
        for dy in range(3):
            for dx in range(3):
                planes[k] = np.transpose(xp[:, ci, dy:dy + 32, dx:dx + 32],
                                         (1, 2, 0))
                k += 1
    out = []
    rem = planes
    for _ in range(N_PLANES):
        p = rem.astype(ml_dtypes.bfloat16)
        out.append(p)
        rem = rem - p.astype(np.float32)
    return np.ascontiguousarray(np.concatenate(out, axis=0))


def host_pack(inputs):
    """Build the per-core DRAM input dicts (weights replicated)."""
    f8 = ml_dtypes.float8_e4m3
    wc = {}
    # L1 weights: [64,3,3,3] -> lhsT [27,64], replicated per plane
    t = np.transpose(_qint(inputs["w1"]), (1, 2, 3, 0)).reshape(27, 64)
    wc["w1sb"] = np.ascontiguousarray(
        np.concatenate([t] * N_PLANES, axis=0).astype(ml_dtypes.bfloat16))
    sb = _scale_bias(inputs["g1"], inputs["b1"], 7.0)  # [64, 2]
    wc["sb1"] = np.ascontiguousarray(
        np.concatenate([sb, sb], axis=0).reshape(128, 1, 2))
    # L2: halves block-diagonal [128, 10, 128]
    wq = _qint(inputs["w2"])  # [64co, 64ci, 3, 3]
    w2p = np.zeros((128, 10, 128), np.float32)
    for ti, (dy, dx) in enumerate(TAPS):
        blk = wq[:, :, dy, dx].T  # [ci, co]
        w2p[0:64, ti, 0:64] = blk
        w2p[64:128, ti, 64:128] = blk
    wc["w2sb"] = np.ascontiguousarray(w2p.astype(f8))
    sb = _scale_bias(inputs["g2"], inputs["b2"], 49.0)
    wc["sb2"] = np.ascontiguousarray(
        np.concatenate([sb, sb], axis=0).reshape(128, 1, 2))
    # L3: dx-packed virtual taps [128, 6, 128]
    wq = _qint(inputs["w3"])  # [128co, 64ci, 3, 3]
    w3p = np.zeros((128, 6, 128), np.float32)
    for dy in range(3):
        for oi, o in enumerate((0, 2)):
            v = 2 * dy + oi
            w3p[0:64, v, :] = wq[:, :, dy, o].T
            if o == 0:
                w3p[64:128, v, :] = wq[:, :, dy, 1].T
    wc["w3sb"] = np.ascontiguousarray(w3p.astype(f8))
    wc["sb3"] = np.ascontiguousarray(
        _scale_bias(inputs["g3"], inputs["b3"], 49.0).reshape(128, 1, 2))
    # L4: [128, 10, 128]
    wq = _qint(inputs["w4"])  # [128, 128, 3, 3]
    w4p = np.zeros((128, 10, 128), np.float32)
    for ti, (dy, dx) in enumerate(TAPS):
        w4p[:, ti, :] = wq[:, :, dy, dx].T
    wc["w4sb"] = np.ascontiguousarray(w4p.astype(f8))
    wc["sb4"] = np.ascontiguousarray(
        _scale_bias(inputs["g4"], inputs["b4"], 49.0).reshape(128, 1, 2))
    # L5: [128, 10, 256]
    wq = _qint(inputs["w5"])  # [256, 128, 3, 3]
    w5p = np.zeros((128, 10, 256), np.float32)
    for ti, (dy, dx) in enumerate(TAPS):
        w5p[:, ti, :] = wq[:, :, dy, dx].T
    wc["w5sb"] = np.ascontiguousarray(w5p.astype(f8))
    wc["sb5"] = np.ascontiguousarray(
        _scale_bias(inputs["g5"], inputs["b5"], 49.0).reshape(2, 128, 2)
        .transpose(1, 0, 2))
    # L6: [128, 9, 2, 256] cig-paired: slots 0-3 pair taps (2p,2p+1) of
    # cig0, slots 4-7 the same of cig1, slot 8 pairs (t8@cig0, t8@cig1).
    wq = _qint(inputs["w6"])  # [256, 256, 3, 3]
    w6p = np.zeros((128, 9, 2, 256), np.float32)
    for cig in range(2):
        for p in range(4):
            (dya, dxa), (dyb, dxb) = TAPS[2 * p], TAPS[2 * p + 1]
            w6p[:, 4 * cig + p, 0, :] = wq[:, cig * 128:(cig + 1) * 128,
                                           dya, dxa].T
            w6p[:, 4 * cig + p, 1, :] = wq[:, cig * 128:(cig + 1) * 128,
                                           dyb, dxb].T
    w6p[:, 8, 0, :] = wq[:, 0:128, 2, 2].T
    w6p[:, 8, 1, :] = wq[:, 128:256, 2, 2].T
    wc["w6sb"] = np.ascontiguousarray(w6p.astype(f8))
    wc["sb6"] = np.ascontiguousarray(
        _scale_bias(inputs["g6"], inputs["b6"], 49.0).reshape(2, 128, 2)
        .transpose(1, 0, 2))
    # FC1 [512, 4096]: k=(c,y,x), c=cig*128+p  -> [128, (cig,16,512)]
    t = _qint(inputs["wf1"]).T.reshape(2, 128, 16, 512)
    wc["wf1sb"] = np.ascontiguousarray(
        np.transpose(t, (1, 0, 2, 3)).reshape(128, 2 * 16 * 512).astype(f8))
    wc["sbf1"] = np.ascontiguousarray(_scale_bias(
        inputs["gf1"], inputs["bf1"], 49.0).reshape(4, 128, 2).transpose(
        1, 0, 2))
    # FC2 [10, 512] -> [128, (4,10)]
    t = _qint(inputs["wf2"]).T.reshape(4, 128, 10)
    wc["wf2sb"] = np.ascontiguousarray(
        np.transpose(t, (1, 0, 2)).reshape(128, 40).astype(f8))
    wc["sbf2"] = _scale_bias(inputs["gf2"], inputs["bf2"], 49.0)

    x = np.asarray(inputs["x"], np.float32)
    maps = []
    for c in range(N_CORES):
        m = dict(wc)
        m["xcol"] = _im2col_bf16(x[c * B:(c + 1) * B])
        maps.append(m)
    return maps


# ----------------------------------------------------------------------------
# Bass program
# ----------------------------------------------------------------------------

def build_nc():
    import concourse.bacc as bacc
    import concourse.mybir as mybir
    import concourse.tile as tile

    dt = mybir.dt
    AF = mybir.ActivationFunctionType
    OP = mybir.AluOpType
    FP8 = dt.float8e4
    DR = mybir.MatmulPerfMode.DoubleRow

    nc = bacc.Bacc("TRN2", target_bir_lowering=False, debug=False)

    xcold = nc.dram_tensor("xcol", [K1, 32, 32, B], dt.bfloat16,
                           kind="ExternalInput")
    w1d = nc.dram_tensor("w1sb", [K1, 64], dt.bfloat16, kind="ExternalInput")
    w2d = nc.dram_tensor("w2sb", [128, 10, 128], FP8, kind="ExternalInput")
    w3d = nc.dram_tensor("w3sb", [128, 6, 128], FP8, kind="ExternalInput")
    w4d = nc.dram_tensor("w4sb", [128, 10, 128], FP8, kind="ExternalInput")
    w5d = nc.dram_tensor("w5sb", [128, 10, 256], FP8, kind="ExternalInput")
    w6d = nc.dram_tensor("w6sb", [128, 9, 2, 256], FP8, kind="ExternalInput")
    sbd = {}
    sbshape = {1: [128, 1, 2], 2: [128, 1, 2], 3: [128, 1, 2],
               4: [128, 1, 2], 5: [128, 2, 2], 6: [128, 2, 2]}
    for i in range(1, 7):
        sbd[i] = nc.dram_tensor(f"sb{i}", sbshape[i], dt.float32,
                                kind="ExternalInput")
    wf1d = nc.dram_tensor("wf1sb", [128, 2 * 16 * 512], FP8,
                          kind="ExternalInput")
    sbf1d = nc.dram_tensor("sbf1", [128, 4, 2], dt.float32,
                           kind="ExternalInput")
    wf2d = nc.dram_tensor("wf2sb", [128, 40], FP8, kind="ExternalInput")
    sbf2d = nc.dram_tensor("sbf2", [10, 2], dt.float32, kind="ExternalInput")
    outd = nc.dram_tensor("out", [B, 10], dt.float32, kind="ExternalOutput")

    with tile.TileContext(nc) as tc:
        # ------------- persistent weights (tiles only, DMAs below) --------
        wp_cm = tc.tile_pool(name="weights", bufs=1)
        wp = wp_cm.__enter__()
        w1 = wp.tile([K1, 64], dt.bfloat16, tag="w1")
        wsb = {}
        for i, shape in ((2, [128, 10, 128]), (3, [128, 6, 128]),
                         (4, [128, 10, 128]), (5, [128, 10, 256]),
                         (6, [128, 9, 2, 256])):
            t = wp.tile(shape, FP8, tag=f"w{i}")
            wsb[i] = t
        sbt = {}
        sbt0 = {}
        for i in range(1, 7):
            t0 = wp.tile(sbshape[i], dt.float32, tag=f"s{i}d")
            sbt0[i] = t0
            t = wp.tile(sbshape[i], dt.float32, tag=f"s{i}")
            sbt[i] = t
        wf2 = wp.tile([128, 40], FP8, tag="wf2")
        sbf2t = wp.tile([10, 2], dt.float32, tag="sf2d")
        sbf2 = wp.tile([10, 2], dt.float32, tag="sf2")

        def zero_border(A, Hp, eng=None):
            e = eng or nc.gpsimd
            e.memset(A[:, 0, :, :], 0.0)
            e.memset(A[:, Hp - 1, :, :], 0.0)
            e.memset(A[:, 1:Hp - 1, 0, :], 0.0)
            e.memset(A[:, 1:Hp - 1, Hp - 1, :], 0.0)

        def with_pair(ap, delta):
            APc = type(ap)
            pairs = list(ap.ap)
            return APc(ap.tensor, ap.offset,
                       [pairs[0], [delta, 2]] + list(pairs[1:]))

        def dr_group(ps_ap, wt, co_sl, base_fn, deltas, npairs, extra=None):
            """Accumulate npairs DR matmuls (+ optional extra groups)."""
            for p in range(npairs):
                nc.tensor.matmul(ps_ap, wt[:, 2 * p:2 * p + 2, co_sl],
                                 with_pair(base_fn(p), deltas[p]),
                                 start=(p == 0), stop=(extra is None
                                                       and p == npairs - 1),
                                 perf_mode=DR)
            if extra is not None:
                wt2, base_fn2, deltas2 = extra
                for p in range(npairs):
                    nc.tensor.matmul(ps_ap, wt2[:, 2 * p:2 * p + 2, co_sl],
                                     with_pair(base_fn2(p), deltas2[p]),
                                     start=False, stop=(p == npairs - 1),
                                     perf_mode=DR)

        # ------------- Layer 1: K=81 im2col conv, x-halves in PSUM --------
        # PSUM partitions 0-63 = left 16 output cols, 64-127 = right 16.
        # The fp8 write then fills A2's lower (channels, x) AND upper
        # (x+16 view) halves in one 128-lane op; only two boundary columns
        # (upper col0 = real x15, lower col17 = real x16) need patch DMAs.
        pa2_cm = tc.tile_pool(name="A2", bufs=1)
        pa2 = pa2_cm.__enter__()
        A2 = pa2.tile([128, 34, 18, B], FP8, tag="A2")

        # xcol staged fully in SBUF: 8 upfront chunk DMAs, 3 queues
        pxc_cm = tc.tile_pool(name="l1mov", bufs=8)
        pxc = pxc_cm.__enter__()
        chunks = []
        chunk_engs = (nc.sync, nc.scalar, nc.gpsimd)
        for c in range(8):
            ck = pxc.tile([K1, 4, 32, B], dt.bfloat16, tag="chunk")
            chunk_engs[c % 3].dma_start(ck[:], xcold[:, 4 * c:4 * c + 4, :, :])
            chunks.append(ck)
        nc.sync.dma_start(w1[:], w1d[:])
        nc.scalar.dma_start(wsb[2][:], w2d[:])
        nc.sync.dma_start(sbt0[1][:], sbd[1][:])
        nc.scalar.copy(sbt[1][:], sbt0[1][:])
        nc.vector.memset(A2[:, 0, :, :], 0.0)
        nc.vector.memset(A2[:, 33, :, :], 0.0)
        nc.gpsimd.memset(A2[:, 1:33, 0, :], 0.0)
        nc.gpsimd.memset(A2[:, 1:33, 17, :], 0.0)
        with (tc.tile_pool(name="l1ps", bufs=2, space="PSUM") as pps,
              tc.tile_pool(name="l1z", bufs=3) as pz):
            for q in range(16):  # y-pairs
                ck = chunks[q // 2]
                ps = pps.tile([128, 4, 512], dt.float32, tag="ps")
                for yy in range(2):
                    yc = 2 * (q % 2) + yy
                    for g in range(2):
                        nc.tensor.matmul(ps[0:64, 2 * yy + g, :], w1[:],
                                         ck[:, yc, g * 8:g * 8 + 8, :],
                                         start=True, stop=True)
                        nc.tensor.matmul(ps[64:128, 2 * yy + g, :], w1[:],
                                         ck[:, yc, 16 + g * 8:
                                            16 + g * 8 + 8, :],
                                         start=True, stop=True)
                z7 = pz.tile([128, 4, 512], dt.float32, tag="z")
                nc.scalar.activation(z7[:], ps[:], AF.Relu,
                                     bias=sbt[1][:, 0, 1:2],
                                     scale=sbt[1][:, 0, 0:1])
                zf = z7[:].rearrange("p a b -> p (a b)")
                nc.vector.tensor_scalar(zf, zf, MAGIC, MAGIC + 7.0,
                                        OP.add, OP.min)
                for yy in range(2):
                    nc.gpsimd.tensor_scalar(
                        A2[:, 1 + 2 * q + yy, 1:17, :].rearrange(
                            "p x b -> p (x b)"),
                        z7[:, 2 * yy:2 * yy + 2, :].rearrange(
                            "p g b -> p (g b)"),
                        MAGIC, None, OP.subtract)
        # boundary patch columns (after all xcol chunks: keep queues clean)
        for k in range(4):
            r0 = 1 + 8 * k
            nc.gpsimd.dma_start(A2[64:128, r0:r0 + 8, 0:1, :],
                                A2[0:64, r0:r0 + 8, 16:17, :])
            nc.gpsimd.dma_start(A2[0:64, r0:r0 + 8, 17:18, :],
                                A2[64:128, r0:r0 + 8, 1:2, :])
        # stream the remaining weights behind the L1 loads
        nc.scalar.dma_start(wsb[3][:], w3d[:])
        nc.scalar.dma_start(wsb[4][:], w4d[:])
        nc.sync.dma_start(wsb[5][:], w5d[:])
        nc.sync.dma_start(wsb[6][:], w6d[:])
        for i in range(2, 7):
            nc.sync.dma_start(sbt0[i][:], sbd[i][:])
            nc.scalar.copy(sbt[i][:], sbt0[i][:])
        nc.sync.dma_start(wf2[:], wf2d[:])
        nc.sync.dma_start(sbf2t[:], sbf2d[:])
        nc.scalar.copy(sbf2[:], sbf2t[:])
        pxc_cm.__exit__(None, None, None)  # free the xcol staging space
        fcw_cm = tc.tile_pool(name="fcw", bufs=1)
        fcw = fcw_cm.__enter__()
        wf1 = fcw.tile([128, 2 * 16 * 512], FP8, tag="wf1")
        sbf1t = fcw.tile([128, 4, 2], dt.float32, tag="sf1d")
        sbf1 = fcw.tile([128, 4, 2], dt.float32, tag="sf1")
        nc.scalar.dma_start(sbf1t[:], sbf1d[:])
        nc.scalar.copy(sbf1[:], sbf1t[:])

        # ------------- Layer 2 (64ch 32x32, halves, pool -> 16) -----------
        pa3_cm = tc.tile_pool(name="A3", bufs=1)
        pa3 = pa3_cm.__enter__()
        A3 = pa3.tile([128, 18, 18, B], FP8, tag="A3")
        zero_border(A3, 18)
        d2 = _pair_deltas(18)
        with (tc.tile_pool(name="c2ps", bufs=3, space="PSUM") as pps,
              tc.tile_pool(name="c2z", bufs=2) as pz,
              tc.tile_pool(name="c2t", bufs=2) as pt):
            w2, sb2 = wsb[2], sbt[2]
            for yo in range(16):
                z7 = pz.tile([128, 2, 2, 8, B], dt.float32, tag="z")
                for yy in range(2):
                    y = 2 * yo + yy
                    ps = pps.tile([128, 2, 512], dt.float32, tag="ps")
                    for xh in range(2):
                        x0 = 8 * xh

                        def mkbase(p, _y=y, _x0=x0):
                            dy, dx = TAPS[2 * p] if p < 4 else TAPS[8]
                            return A2[:, _y + dy, _x0 + dx:_x0 + dx + 8, :]
                        dr_group(ps[:, xh, :], w2, slice(0, 128), mkbase,
                                 d2, 5)
                    nc.scalar.activation(
                        z7[:, yy, :, :, :].rearrange("p a x b -> p (a x b)")
                        .rearrange("p (a b) -> p a b", b=512),
                        ps[:], AF.Relu, bias=sb2[:, 0, 1:2],
                        scale=sb2[:, 0, 0:1])
                zx = pt.tile([128, 2, 2, 4, B], dt.float32, tag="zx")
                for yy in range(2):
                    nc.vector.tensor_tensor(
                        zx[:, yy, :, :, :], z7[:, yy, :, 0::2, :],
                        z7[:, yy, :, 1::2, :], OP.max)
                zp = pt.tile([128, 2, 4, B], dt.float32, tag="zp")
                nc.vector.tensor_tensor(zp[:], zx[:, 0, :, :, :],
                                        zx[:, 1, :, :, :], OP.max)
                zpf = zp[:].rearrange("p a x b -> p (a x b)")
                nc.vector.tensor_scalar(zpf, zpf, MAGIC, MAGIC + 7.0,
                                        OP.add, OP.min)
                a3t = pt.tile([128, 2, 4, B], FP8, tag="a3t")
                nc.gpsimd.tensor_scalar(
                    a3t[:].rearrange("p a x b -> p (a x b)"), zpf, MAGIC,
                    None, OP.subtract)
                nc.sync.dma_start(A3[0:64, 1 + yo, 1:9, :],
                                  a3t[0:64].rearrange("p a x b -> p (a x) b"))
                nc.sync.dma_start(A3[0:64, 1 + yo, 9:17, :],
                                  a3t[64:128].rearrange(
                                      "p a x b -> p (a x) b"))
                # x+1 dup copy for L3 dx-packing, 4 chunks
                if yo in (2, 7, 12, 15):
                    r0, r1 = {2: (0, 4), 7: (4, 9), 12: (9, 14),
                              15: (14, 18)}[yo]
                    nc.sync.dma_start(A3[64:128, r0:r1, 0:17, :],
                                      A3[0:64, r0:r1, 1:18, :])

        nc.gpsimd.dma_start(wf1[:], wf1d[:])

        # ------------- Layer 3 (64 -> 128, 16x16, dx-packed) --------------
        pa4_cm = tc.tile_pool(name="A4", bufs=1)
        pa4 = pa4_cm.__enter__()
        A4 = pa4.tile([128, 18, 18, B], FP8, tag="A4")
        zero_border(A4, 18)
        with (tc.tile_pool(name="c3ps", bufs=3, space="PSUM") as pps,
              tc.tile_pool(name="c3z", bufs=3) as pz):
            w3, sb3 = wsb[3], sbt[3]
            for y in range(16):
                ps = pps.tile([128, 2, 512], dt.float32, tag="ps")
                for xh in range(2):
                    x0 = 8 * xh
                    for dy in range(3):
                        base = A3[:, y + dy, x0:x0 + 8, :]
                        nc.tensor.matmul(
                            ps[:, xh, :], w3[:, 2 * dy:2 * dy + 2, :],
                            with_pair(base, 2 * B), start=(dy == 0),
                            stop=(dy == 2), perf_mode=DR)
                z7 = pz.tile([128, 2, 8, B], dt.float32, tag="z")
                nc.scalar.activation(
                    z7[:].rearrange("p a x b -> p (a x b)")
                    .rearrange("p (a b) -> p a b", b=512),
                    ps[:], AF.Relu, bias=sb3[:, 0, 1:2], scale=sb3[:, 0, 0:1])
                zf = z7[:].rearrange("p a x b -> p (a x b)")
                nc.vector.tensor_scalar(zf, zf, MAGIC, MAGIC + 7.0,
                                        OP.add, OP.min)
                nc.gpsimd.tensor_scalar(
                    A4[:, 1 + y, 1:17, :].rearrange("p x b -> p (x b)"),
                    zf, MAGIC, None, OP.subtract)

        # ------------- Layer 4 (128 -> 128, 16x16, pool -> 8) -------------
        pa5_cm = tc.tile_pool(name="A5", bufs=1)
        pa5 = pa5_cm.__enter__()
        A5 = pa5.tile([128, 10, 10, B], FP8, tag="A5")
        zero_border(A5, 10)
        d4 = _pair_deltas(18)
        with (tc.tile_pool(name="c4ps", bufs=3, space="PSUM") as pps,
              tc.tile_pool(name="c4z", bufs=2) as pz,
              tc.tile_pool(name="c4t", bufs=2) as pt):
            w4, sb4 = wsb[4], sbt[4]
            for yo in range(8):
                z7 = pz.tile([128, 2, 2, 8, B], dt.float32, tag="z")
                for yy in range(2):
                    y = 2 * yo + yy
                    ps = pps.tile([128, 2, 512], dt.float32, tag="ps")
                    for xh in range(2):
                        x0 = 8 * xh

                        def mkbase(p, _y=y, _x0=x0):
                            dy, dx = TAPS[2 * p] if p < 4 else TAPS[8]
                            return A4[:, _y + dy, _x0 + dx:_x0 + dx + 8, :]
                        dr_group(ps[:, xh, :], w4, slice(0, 128), mkbase,
                                 d4, 5)
                    nc.scalar.activation(
                        z7[:, yy, :, :, :].rearrange("p a x b -> p (a x b)")
                        .rearrange("p (a b) -> p a b", b=512),
                        ps[:], AF.Relu, bias=sb4[:, 0, 1:2],
                        scale=sb4[:, 0, 0:1])
                zx = pt.tile([128, 2, 2, 4, B], dt.float32, tag="zx")
                for yy in range(2):
                    nc.vector.tensor_tensor(
                        zx[:, yy, :, :, :], z7[:, yy, :, 0::2, :],
                        z7[:, yy, :, 1::2, :], OP.max)
                zp = pt.tile([128, 2, 4, B], dt.float32, tag="zp")
                nc.vector.tensor_tensor(zp[:], zx[:, 0, :, :, :],
                                        zx[:, 1, :, :, :], OP.max)
                zpf = zp[:].rearrange("p a x b -> p (a x b)")
                nc.vector.tensor_scalar(zpf, zpf, MAGIC, MAGIC + 7.0,
                                        OP.add, OP.min)
                nc.gpsimd.tensor_scalar(
                    A5[:, 1 + yo, 1:9, :].rearrange("p x b -> p (x b)"),
                    zpf, MAGIC, None, OP.subtract)

        # ------------- Layer 5 (128 -> 256, 8x8) --------------------------
        pa6_cm = tc.tile_pool(name="A6", bufs=1)
        pa6 = pa6_cm.__enter__()
        A6 = pa6.tile([128, 2, 10, 10, B], FP8, tag="A6")
        nc.gpsimd.memset(A6[:, :, 0, :, :], 0.0)
        nc.gpsimd.memset(A6[:, :, 9, :, :], 0.0)
        nc.gpsimd.memset(A6[:, :, 1:9, 0, :], 0.0)
        nc.gpsimd.memset(A6[:, :, 1:9, 9, :], 0.0)
        d5 = _pair_deltas(10)
        with (tc.tile_pool(name="c5ps", bufs=3, space="PSUM") as pps,
              tc.tile_pool(name="c5z", bufs=3) as pz):
            w5, sb5 = wsb[5], sbt[5]
            for y in range(8):
                ps = pps.tile([128, 2, 512], dt.float32, tag="ps")
                for ct in range(2):
                    def mkbase(p, _y=y):
                        dy, dx = TAPS[2 * p] if p < 4 else TAPS[8]
                        return A5[:, _y + dy, dx:dx + 8, :]
                    dr_group(ps[:, ct, :], w5,
                             slice(ct * 128, ct * 128 + 128), mkbase, d5, 5)
                z7 = pz.tile([128, 2, 8, B], dt.float32, tag="z")
                for ct in range(2):
                    nc.scalar.activation(
                        z7[:, ct, :, :].rearrange("p x b -> p (x b)"),
                        ps[:, ct, :],
                        AF.Relu, bias=sb5[:, ct, 1:2], scale=sb5[:, ct, 0:1])
                zf = z7[:].rearrange("p c x b -> p (c x b)")
                nc.vector.tensor_scalar(zf, zf, MAGIC, MAGIC + 7.0,
                                        OP.add, OP.min)
                for ct in range(2):
                    nc.gpsimd.tensor_scalar(
                        A6[:, ct, 1 + y, 1:9, :].rearrange(
                            "p x b -> p (x b)"),
                        z7[:, ct, :, :].rearrange("p x b -> p (x b)"),
                        MAGIC, None, OP.subtract)

        # ------------- Layer 6 (256 -> 256, 8x8, pool -> 4) ---------------
        pa7_cm = tc.tile_pool(name="A7", bufs=1)
        pa7 = pa7_cm.__enter__()
        A7 = pa7.tile([128, 2, 4, 4, B], FP8, tag="A7")  # unpadded, feeds FC
        d6 = _pair_deltas(10)
        CIG = 10 * 10 * B  # element offset between the two ci-groups of A6
        with (tc.tile_pool(name="c6ps", bufs=3, space="PSUM") as pps,
              tc.tile_pool(name="c6z", bufs=2) as pz,
              tc.tile_pool(name="c6t", bufs=2) as pt):
            w6, sb6 = wsb[6], sbt[6]
            for yo in range(4):
                z7 = pz.tile([128, 2, 2, 8, B], dt.float32, tag="z")
                for yy in range(2):
                    y = 2 * yo + yy
                    ps = pps.tile([128, 2, 512], dt.float32, tag="ps")
                    for ct in range(2):
                        co_sl = slice(ct * 128, ct * 128 + 128)
                        for p in range(9):
                            if p < 8:
                                cig, pp = p // 4, p % 4
                                dy, dx = TAPS[2 * pp]
                                base = A6[:, cig, y + dy, dx:dx + 8, :]
                                delta = d6[pp]
                            else:
                                base = A6[:, 0, y + 2, 2:2 + 8, :]
                                delta = CIG
                            nc.tensor.matmul(
                                ps[:, ct, :], w6[:, p, :, co_sl],
                                with_pair(base, delta),
                                start=(p == 0), stop=(p == 8), perf_mode=DR)
                    for ct in range(2):
                        nc.scalar.activation(
                            z7[:, yy, ct, :, :].rearrange(
                                "p x b -> p (x b)"),
                            ps[:, ct, :],
                            AF.Relu, bias=sb6[:, ct, 1:2],
                            scale=sb6[:, ct, 0:1])
                zx = pt.tile([128, 2, 2, 4, B], dt.float32, tag="zx")
                for yy in range(2):
                    nc.vector.tensor_tensor(
                        zx[:, yy, :, :, :], z7[:, yy, :, 0::2, :],
                        z7[:, yy, :, 1::2, :], OP.max)
                zp = pt.tile([128, 2, 4, B], dt.float32, tag="zp")
                nc.vector.tensor_tensor(zp[:], zx[:, 0, :, :, :],
                                        zx[:, 1, :, :, :], OP.max)
                zpf = zp[:].rearrange("p c x b -> p (c x b)")
                nc.vector.tensor_scalar(zpf, zpf, MAGIC, MAGIC + 7.0,
                                        OP.add, OP.min)
                for ct in range(2):
                    nc.gpsimd.tensor_scalar(
                        A7[:, ct, yo, :, :].rearrange("p x b -> p (x b)"),
                        zp[:, ct, :, :].rearrange("p x b -> p (x b)"),
                        MAGIC, None, OP.subtract)

        # ------------- FC1 (4096 -> 512) ----------------------------------
        pa8_cm = tc.tile_pool(name="A8", bufs=1)
        pa8 = pa8_cm.__enter__()
        A8 = pa8.tile([128, 4, B], FP8, tag="A8")
        with (tc.tile_pool(name="f1ps", bufs=4, space="PSUM") as pps,
              tc.tile_pool(name="f1t", bufs=4) as pt):
            for ct in range(4):
                ps = pps.tile([128, B], dt.float32, tag="ps")
                k = 0
                for cig in range(2):
                    for px in range(16):
                        wo = (cig * 16 + px) * 512 + ct * 128
                        nc.tensor.matmul(ps[:], wf1[:, wo:wo + 128],
                                         A7[:, cig, px // 4, px % 4, :],
                                         start=(k == 0), stop=(k == 31))
                        k += 1
                z7 = pt.tile([128, B], dt.float32, tag="z")
                nc.scalar.activation(z7[:], ps[:], AF.Relu,
                                     bias=sbf1[:, ct, 1:2],
                                     scale=sbf1[:, ct, 0:1])
                nc.vector.tensor_scalar(z7[:], z7[:], MAGIC, MAGIC + 7.0,
                                        OP.add, OP.min)
                nc.gpsimd.tensor_scalar(A8[:, ct, :], z7[:], MAGIC, None,
                                        OP.subtract)

        # ------------- FC2 (512 -> 10), signed output ---------------------
        with (tc.tile_pool(name="f2ps", bufs=1, space="PSUM") as pps,
              tc.tile_pool(name="f2t", bufs=1) as pt):
            ps = pps.tile([10, B], dt.float32, tag="ps")
            for kt in range(4):
                nc.tensor.matmul(ps[:], wf2[:, kt * 10:(kt + 1) * 10],
                                 A8[:, kt, :], start=(kt == 0), stop=(kt == 3))
            z7 = pt.tile([10, B], dt.float32, tag="z")
            nc.vector.tensor_scalar(z7[:], ps[:], sbf2[:, 0:1], sbf2[:, 1:2],
                                    OP.mult, OP.add)
            r = pt.tile([10, B], dt.float32, tag="r")
            nc.vector.tensor_scalar(r[:], z7[:], MAGIC, MAGIC - 7.0,
                                    OP.add, OP.max)  # RNE + lower clamp
            r2 = pt.tile([10, B], dt.float32, tag="r2")
            nc.vector.tensor_scalar(r2[:], r[:], MAGIC + 7.0, MAGIC,
                                    OP.min, OP.subtract)
            fin = pt.tile([10, B], dt.float32, tag="fin")
            nc.vector.tensor_scalar(fin[:], r2[:], 1.0 / 7.0,
                                    None, OP.mult)
            nc.sync.dma_start(outd[:].rearrange("b c -> c b"), fin[:])
        for cm in (pa8_cm, pa7_cm, pa6_cm, pa5_cm, pa4_cm, pa3_cm, fcw_cm,
                   pa2_cm):
            cm.__exit__(None, None, None)
        wp_cm.__exit__(None, None, None)

    nc.compile()
    return nc


# ----------------------------------------------------------------------------
# Entry point
# ----------------------------------------------------------------------------

_NC_CACHE = {}
LAST_RESULTS = None  # BassKernelResults of the most recent run (for test.py)


def kernel(**inputs):
    global LAST_RESULTS
    from concourse.bass_utils import run_bass_kernel_spmd
    if "nc" not in _NC_CACHE:
        _NC_CACHE["nc"] = build_nc()
    nc = _NC_CACHE["nc"]
    in_maps = host_pack(inputs)
    res = run_bass_kernel_spmd(nc, in_maps, list(range(N_CORES)))
    LAST_RESULTS = res
    outs = [res.results[c]["out"] for c in range(N_CORES)]
    return np.concatenate(outs, axis=0).astype(np.float32)


# revision 8
# speedup vs baseline: 3.5021x; 3.5021x over previous
"""Trainium2 Bass kernel for nn_IntegerCifar10Net (quantized VGG-ish CNN).

Data parallel over 8 NeuronCores, B=64 images/core.

v3: engine-balanced quant chain + upfront chunked xcol streaming.

Layer matmul schemes (unchanged from v2 except L6):
  L1 : exact 3-plane bf16 im2col (K=81), co=64; x-halves to PSUM partitions
       0-63 / 64-127; quantized row writes A2 lower+upper halves in one op.
  L2 : "halves" trick - PSUM partitions 0-63 = left 16 output cols,
       64-127 = right 16; block-diagonal weights, 5 DR matmuls per bank.
  L3 : dx-packing (x+1 dup in upper partitions): 3 DR matmuls per bank.
  L4/L5: plain 9 taps -> 5 DR matmuls per bank.
  L6 : 9 DR matmuls per bank - the two odd 9th taps of the two ci-groups
       share one DR pair (cig-pairing) instead of 2 zero-padded pairs.

Quant chain per bank, balanced across Scalar(Act) and Vector(DVE) (the
GpSimd/Pool engine only has slow Q7-ucode elementwise ops on TRN2, and
they also stall DVE via the shared SBUF port - measured 10.6us/op):
  Act : z = relu(psum*scale + bias)                  [PSUM -> SBUF f32]
  DVE : u = min(z + MAGIC, MAGIC+7)  (in-place)      [RNE round + clamp]
  Act/DVE : a = u - MAGIC -> fp8 (Copy activation with immediate bias
  -MAGIC on Act, tensor_scalar on DVE; split to balance engine load)
Max-pool layers run the pairwise maxes on DVE over pre-round z.

xcol is staged fully in SBUF via 8 upfront chunk DMAs (4 rows each)
round-robined over the sync/scalar/gpsimd queues; weights stream behind.
"""

import sys
import numpy as np

sys.path.insert(0, "/opt/trn_rl_repo")

import ml_dtypes

N_CORES = 8
B = 64  # images per core
MAGIC = 12582912.0  # 1.5 * 2^23 : RNE rounding magic for |v| < 2^22
N_PLANES = 3  # bf16 planes for exact L1 input (hi/mid/lo)
K1 = 27 * N_PLANES

# tap pair schedule for 3x3 convs: pairs of taps t=(dy,dx) row-major,
# 10th tap is zero-weight padding with moving delta -B (always in bounds)
TAPS = [(dy, dx) for dy in range(3) for dx in range(3)]


def _pair_deltas(W):
    """Moving-AP element deltas between the two taps of each DR pair."""
    ds = []
    for p in range(4):
        (dya, dxa), (dyb, dxb) = TAPS[2 * p], TAPS[2 * p + 1]
        ds.append(((dyb - dya) * W + (dxb - dxa)) * B)
    ds.append(-B)  # pad pair: (t8, zero-weight tap at x-1)
    return ds


# ----------------------------------------------------------------------------
# Host-side packing
# ----------------------------------------------------------------------------

def _qint(w):
    """round(clip(w,-1,1)*7) as float32 integers, matching jax fp32 chain."""
    w = np.asarray(w, np.float32)
    return np.round(np.clip(w, -1.0, 1.0) * np.float32(7.0)).astype(np.float32)


def _scale_bias(g, b, denom):
    # z7 = conv_int * (7*g/denom) + 7*b, constants in f64 then rounded to f32
    s = (7.0 * np.asarray(g, np.float64) / denom).astype(np.float32)
    bt = (7.0 * np.asarray(b, np.float64)).astype(np.float32)
    return np.ascontiguousarray(np.stack([s, bt], axis=1))  # [co, 2] f32


def _im2col_bf16(x):
    """x [B,3,32,32] f32 -> [K1, 32, 32, B] bf16 (N_PLANES x 27 rows)."""
    Bc = x.shape[0]
    xp = np.zeros((Bc, 3, 34, 34), np.float32)
    xp[:, :, 1:33, 1:33] = x
    planes = np.empty((27, 32, 32, Bc), np.float32)
    k = 0
    for ci in range(3):
        for dy in range(3):
            for dx in range(3):
                planes[k] = np.transpose(xp[:, ci, dy:dy + 32, dx:dx + 32],
                                         (1, 2, 0))
                k += 1
    out = []
    rem = planes
    for _ in range(N_PLANES):
        p = rem.astype(ml_dtypes.bfloat16)
        out.append(p)
        rem = rem - p.astype(np.float32)
    return np.ascontiguousarray(np.concatenate(out, axis=0))


def host_pack(inputs):
    """Build the per-core DRAM input dicts (weights replicated)."""
    f8 = ml_dtypes.float8_e4m3
    wc = {}
    # L1 weights: [64,3,3,3] -> lhsT [27,64], replicated per plane
    t = np.transpose(_qint(inputs["w1"]), (1, 2, 3, 0)).reshape(27, 64)
    wc["w1sb"] = np.ascontiguousarray(
        np.concatenate([t] * N_PLANES, axis=0).astype(ml_dtypes.bfloat16))
    sb = _scale_bias(inputs["g1"], inputs["b1"], 7.0)  # [64, 2]
    wc["sb1"] = np.ascontiguousarray(
        np.concatenate([sb, sb], axis=0).reshape(128, 1, 2))
    # L2: halves block-diagonal [128, 10, 128]
    wq = _qint(inputs["w2"])  # [64co, 64ci, 3, 3]
    w2p = np.zeros((128, 10, 128), np.float32)
    for ti, (dy, dx) in enumerate(TAPS):
        blk = wq[:, :, dy, dx].T  # [ci, co]
        w2p[0:64, ti, 0:64] = blk
        w2p[64:128, ti, 64:128] = blk
    wc["w2sb"] = np.ascontiguousarray(w2p.astype(f8))
    sb = _scale_bias(inputs["g2"], inputs["b2"], 49.0)
    wc["sb2"] = np.ascontiguousarray(
        np.concatenate([sb, sb], axis=0).reshape(128, 1, 2))
    # L3: dx-packed virtual taps [128, 6, 128]
    wq = _qint(inputs["w3"])  # [128co, 64ci, 3, 3]
    w3p = np.zeros((128, 6, 128), np.float32)
    for dy in range(3):
        for oi, o in enumerate((0, 2)):
            v = 2 * dy + oi
            w3p[0:64, v, :] = wq[:, :, dy, o].T
            if o == 0:
                w3p[64:128, v, :] = wq[:, :, dy, 1].T
    wc["w3sb"] = np.ascontiguousarray(w3p.astype(f8))
    wc["sb3"] = np.ascontiguousarray(
        _scale_bias(inputs["g3"], inputs["b3"], 49.0).reshape(128, 1, 2))
    # L4: [128, 10, 128]
    wq = _qint(inputs["w4"])  # [128, 128, 3, 3]
    w4p = np.zeros((128, 10, 128), np.float32)
    for ti, (dy, dx) in enumerate(TAPS):
        w4p[:, ti, :] = wq[:, :, dy, dx].T
    wc["w4sb"] = np.ascontiguousarray(w4p.astype(f8))
    wc["sb4"] = np.ascontiguousarray(
        _scale_bias(inputs["g4"], inputs["b4"], 49.0).reshape(128, 1, 2))
    # L5: [128, 10, 256]
    wq = _qint(inputs["w5"])  # [256, 128, 3, 3]
    w5p = np.zeros((128, 10, 256), np.float32)
    for ti, (dy, dx) in enumerate(TAPS):
        w5p[:, ti, :] = wq[:, :, dy, dx].T
    wc["w5sb"] = np.ascontiguousarray(w5p.astype(f8))
    wc["sb5"] = np.ascontiguousarray(
        _scale_bias(inputs["g5"], inputs["b5"], 49.0).reshape(2, 128, 2)
        .transpose(1, 0, 2))
    # L6: [128, 9, 2, 256] cig-paired: slots 0-3 pair taps (2p,2p+1) of
    # cig0, slots 4-7 the same of cig1, slot 8 pairs (t8@cig0, t8@cig1).
    wq = _qint(inputs["w6"])  # [256, 256, 3, 3]
    w6p = np.zeros((128, 9, 2, 256), np.float32)
    for cig in range(2):
        for p in range(4):
            (dya, dxa), (dyb, dxb) = TAPS[2 * p], TAPS[2 * p + 1]
            w6p[:, 4 * cig + p, 0, :] = wq[:, cig * 128:(cig + 1) * 128,
                                           dya, dxa].T
            w6p[:, 4 * cig + p, 1, :] = wq[:, cig * 128:(cig + 1) * 128,
                                           dyb, dxb].T
    w6p[:, 8, 0, :] = wq[:, 0:128, 2, 2].T
    w6p[:, 8, 1, :] = wq[:, 128:256, 2, 2].T
    wc["w6sb"] = np.ascontiguousarray(w6p.astype(f8))
    wc["sb6"] = np.ascontiguousarray(
        _scale_bias(inputs["g6"], inputs["b6"], 49.0).reshape(2, 128, 2)
        .transpose(1, 0, 2))
    # FC1 [512, 4096]: k=(c,y,x), c=cig*128+p  -> [128, (cig,16,512)]
    t = _qint(inputs["wf1"]).T.reshape(2, 128, 16, 512)
    wc["wf1sb"] = np.ascontiguousarray(
        np.transpose(t, (1, 0, 2, 3)).reshape(128, 2 * 16 * 512).astype(f8))
    wc["sbf1"] = np.ascontiguousarray(_scale_bias(
        inputs["gf1"], inputs["bf1"], 49.0).reshape(4, 128, 2).transpose(
        1, 0, 2))
    # FC2 [10, 512] -> [128, (4,10)]
    t = _qint(inputs["wf2"]).T.reshape(4, 128, 10)
    wc["wf2sb"] = np.ascontiguousarray(
        np.transpose(t, (1, 0, 2)).reshape(128, 40).astype(f8))
    wc["sbf2"] = _scale_bias(inputs["gf2"], inputs["bf2"], 49.0)

    x = np.asarray(inputs["x"], np.float32)
    maps = []
    for c in range(N_CORES):
        m = dict(wc)
        m["xcol"] = _im2col_bf16(x[c * B:(c + 1) * B])
        maps.append(m)
    return maps


# ----------------------------------------------------------------------------
# Bass program
# ----------------------------------------------------------------------------

def build_nc():
    import concourse.bacc as bacc
    import concourse.mybir as mybir
    import concourse.tile as tile

    dt = mybir.dt
    AF = mybir.ActivationFunctionType
    OP = mybir.AluOpType
    FP8 = dt.float8e4
    DR = mybir.MatmulPerfMode.DoubleRow

    nc = bacc.Bacc("TRN2", target_bir_lowering=False, debug=False)

    xcold = nc.dram_tensor("xcol", [K1, 32, 32, B], dt.bfloat16,
                           kind="ExternalInput")
    w1d = nc.dram_tensor("w1sb", [K1, 64], dt.bfloat16, kind="ExternalInput")
    w2d = nc.dram_tensor("w2sb", [128, 10, 128], FP8, kind="ExternalInput")
    w3d = nc.dram_tensor("w3sb", [128, 6, 128], FP8, kind="ExternalInput")
    w4d = nc.dram_tensor("w4sb", [128, 10, 128], FP8, kind="ExternalInput")
    w5d = nc.dram_tensor("w5sb", [128, 10, 256], FP8, kind="ExternalInput")
    w6d = nc.dram_tensor("w6sb", [128, 9, 2, 256], FP8, kind="ExternalInput")
    sbd = {}
    sbshape = {1: [128, 1, 2], 2: [128, 1, 2], 3: [128, 1, 2],
               4: [128, 1, 2], 5: [128, 2, 2], 6: [128, 2, 2]}
    for i in range(1, 7):
        sbd[i] = nc.dram_tensor(f"sb{i}", sbshape[i], dt.float32,
                                kind="ExternalInput")
    wf1d = nc.dram_tensor("wf1sb", [128, 2 * 16 * 512], FP8,
                          kind="ExternalInput")
    sbf1d = nc.dram_tensor("sbf1", [128, 4, 2], dt.float32,
                           kind="ExternalInput")
    wf2d = nc.dram_tensor("wf2sb", [128, 40], FP8, kind="ExternalInput")
    sbf2d = nc.dram_tensor("sbf2", [10, 2], dt.float32, kind="ExternalInput")
    outd = nc.dram_tensor("out", [B, 10], dt.float32, kind="ExternalOutput")

    with tile.TileContext(nc) as tc:
        # ------------- persistent weights (tiles only, DMAs below) --------
        wp_cm = tc.tile_pool(name="weights", bufs=1)
        wp = wp_cm.__enter__()
        w1 = wp.tile([K1, 64], dt.bfloat16, tag="w1")
        wsb = {}
        for i, shape in ((2, [128, 10, 128]), (3, [128, 6, 128]),
                         (4, [128, 10, 128]), (5, [128, 10, 256]),
                         (6, [128, 9, 2, 256])):
            t = wp.tile(shape, FP8, tag=f"w{i}")
            wsb[i] = t
        sbt = {}
        sbt0 = {}
        for i in range(1, 7):
            t0 = wp.tile(sbshape[i], dt.float32, tag=f"s{i}d")
            sbt0[i] = t0
            t = wp.tile(sbshape[i], dt.float32, tag=f"s{i}")
            sbt[i] = t
        wf2 = wp.tile([128, 40], FP8, tag="wf2")
        sbf2t = wp.tile([10, 2], dt.float32, tag="sf2d")
        sbf2 = wp.tile([10, 2], dt.float32, tag="sf2")

        def zero_border(A, Hp, eng=None):
            e = eng or nc.gpsimd
            e.memset(A[:, 0, :, :], 0.0)
            e.memset(A[:, Hp - 1, :, :], 0.0)
            e.memset(A[:, 1:Hp - 1, 0, :], 0.0)
            e.memset(A[:, 1:Hp - 1, Hp - 1, :], 0.0)

        def with_pair(ap, delta):
            APc = type(ap)
            pairs = list(ap.ap)
            return APc(ap.tensor, ap.offset,
                       [pairs[0], [delta, 2]] + list(pairs[1:]))

        def dr_group(ps_ap, wt, co_sl, base_fn, deltas, npairs, extra=None):
            """Accumulate npairs DR matmuls (+ optional extra groups)."""
            for p in range(npairs):
                nc.tensor.matmul(ps_ap, wt[:, 2 * p:2 * p + 2, co_sl],
                                 with_pair(base_fn(p), deltas[p]),
                                 start=(p == 0), stop=(extra is None
                                                       and p == npairs - 1),
                                 perf_mode=DR)
            if extra is not None:
                wt2, base_fn2, deltas2 = extra
                for p in range(npairs):
                    nc.tensor.matmul(ps_ap, wt2[:, 2 * p:2 * p + 2, co_sl],
                                     with_pair(base_fn2(p), deltas2[p]),
                                     start=False, stop=(p == npairs - 1),
                                     perf_mode=DR)

        # ------------- Layer 1: K=81 im2col conv, x-halves in PSUM --------
        # PSUM partitions 0-63 = left 16 output cols, 64-127 = right 16.
        # The fp8 write then fills A2's lower (channels, x) AND upper
        # (x+16 view) halves in one 128-lane op; only two boundary columns
        # (upper col0 = real x15, lower col17 = real x16) need patch DMAs.
        pa2_cm = tc.tile_pool(name="A2", bufs=1)
        pa2 = pa2_cm.__enter__()
        A2 = pa2.tile([128, 34, 18, B], FP8, tag="A2")

        # xcol staged fully in SBUF: 8 upfront chunk DMAs, 3 queues
        pxc_cm = tc.tile_pool(name="l1mov", bufs=8)
        pxc = pxc_cm.__enter__()
        chunks = []
        chunk_engs = (nc.sync, nc.scalar, nc.gpsimd)
        for c in range(8):
            ck = pxc.tile([K1, 4, 32, B], dt.bfloat16, tag="chunk")
            chunk_engs[c % 3].dma_start(ck[:], xcold[:, 4 * c:4 * c + 4, :, :])
            chunks.append(ck)
        nc.sync.dma_start(w1[:], w1d[:])
        nc.scalar.dma_start(wsb[2][:], w2d[:])
        nc.sync.dma_start(sbt0[1][:], sbd[1][:])
        nc.scalar.copy(sbt[1][:], sbt0[1][:])
        nc.vector.memset(A2[:, 0, :, :], 0.0)
        nc.vector.memset(A2[:, 33, :, :], 0.0)
        nc.gpsimd.memset(A2[:, 1:33, 0, :], 0.0)
        nc.gpsimd.memset(A2[:, 1:33, 17, :], 0.0)
        with (tc.tile_pool(name="l1ps", bufs=2, space="PSUM") as pps,
              tc.tile_pool(name="l1z", bufs=3) as pz):
            for q in range(16):  # y-pairs
                ck = chunks[q // 2]
                ps = pps.tile([128, 4, 512], dt.float32, tag="ps")
                for yy in range(2):
                    yc = 2 * (q % 2) + yy
                    for g in range(2):
                        nc.tensor.matmul(ps[0:64, 2 * yy + g, :], w1[:],
                                         ck[:, yc, g * 8:g * 8 + 8, :],
                                         start=True, stop=True)
                        nc.tensor.matmul(ps[64:128, 2 * yy + g, :], w1[:],
                                         ck[:, yc, 16 + g * 8:
                                            16 + g * 8 + 8, :],
                                         start=True, stop=True)
                z7 = pz.tile([128, 4, 512], dt.float32, tag="z")
                nc.scalar.activation(z7[:], ps[:], AF.Relu,
                                     bias=sbt[1][:, 0, 1:2],
                                     scale=sbt[1][:, 0, 0:1])
                zf = z7[:].rearrange("p a b -> p (a b)")
                nc.vector.tensor_scalar(zf, zf, MAGIC, MAGIC + 7.0,
                                        OP.add, OP.min)
                # fp8 store: rows (yy=0..1) -> A2 rows; yy0+g0 half on DVE,
                # the rest on Act, balancing both engines
                nc.vector.tensor_scalar(
                    A2[:, 1 + 2 * q, 1:9, :].rearrange("p x b -> p (x b)"),
                    z7[:, 0, :], MAGIC, None, OP.subtract)
                nc.scalar.activation(
                    A2[:, 1 + 2 * q, 9:17, :].rearrange("p x b -> p (x b)"),
                    z7[:, 1, :], AF.Copy, bias=-MAGIC)
                nc.scalar.activation(
                    A2[:, 2 + 2 * q, 1:17, :].rearrange("p x b -> p (x b)"),
                    z7[:, 2:4, :].rearrange("p g b -> p (g b)"),
                    AF.Copy, bias=-MAGIC)
        # boundary patch columns (after all xcol chunks: keep queues clean)
        for k in range(4):
            r0 = 1 + 8 * k
            nc.gpsimd.dma_start(A2[64:128, r0:r0 + 8, 0:1, :],
                                A2[0:64, r0:r0 + 8, 16:17, :])
            nc.gpsimd.dma_start(A2[0:64, r0:r0 + 8, 17:18, :],
                                A2[64:128, r0:r0 + 8, 1:2, :])
        # stream the remaining weights behind the L1 loads
        nc.scalar.dma_start(wsb[3][:], w3d[:])
        nc.scalar.dma_start(wsb[4][:], w4d[:])
        nc.sync.dma_start(wsb[5][:], w5d[:])
        nc.sync.dma_start(wsb[6][:], w6d[:])
        for i in range(2, 7):
            nc.sync.dma_start(sbt0[i][:], sbd[i][:])
            nc.scalar.copy(sbt[i][:], sbt0[i][:])
        nc.sync.dma_start(wf2[:], wf2d[:])
        nc.sync.dma_start(sbf2t[:], sbf2d[:])
        nc.scalar.copy(sbf2[:], sbf2t[:])
        pxc_cm.__exit__(None, None, None)  # free the xcol staging space
        fcw_cm = tc.tile_pool(name="fcw", bufs=1)
        fcw = fcw_cm.__enter__()
        wf1 = fcw.tile([128, 2 * 16 * 512], FP8, tag="wf1")
        sbf1t = fcw.tile([128, 4, 2], dt.float32, tag="sf1d")
        sbf1 = fcw.tile([128, 4, 2], dt.float32, tag="sf1")
        nc.scalar.dma_start(sbf1t[:], sbf1d[:])
        nc.scalar.copy(sbf1[:], sbf1t[:])

        # ------------- Layer 2 (64ch 32x32, halves, pool -> 16) -----------
        pa3_cm = tc.tile_pool(name="A3", bufs=1)
        pa3 = pa3_cm.__enter__()
        A3 = pa3.tile([128, 18, 18, B], FP8, tag="A3")
        zero_border(A3, 18)
        d2 = _pair_deltas(18)
        with (tc.tile_pool(name="c2ps", bufs=3, space="PSUM") as pps,
              tc.tile_pool(name="c2z", bufs=2) as pz,
              tc.tile_pool(name="c2t", bufs=2) as pt):
            w2, sb2 = wsb[2], sbt[2]
            for yo in range(16):
                z7 = pz.tile([128, 2, 2, 8, B], dt.float32, tag="z")
                for yy in range(2):
                    y = 2 * yo + yy
                    ps = pps.tile([128, 2, 512], dt.float32, tag="ps")
                    for xh in range(2):
                        x0 = 8 * xh

                        def mkbase(p, _y=y, _x0=x0):
                            dy, dx = TAPS[2 * p] if p < 4 else TAPS[8]
                            return A2[:, _y + dy, _x0 + dx:_x0 + dx + 8, :]
                        dr_group(ps[:, xh, :], w2, slice(0, 128), mkbase,
                                 d2, 5)
                    nc.scalar.activation(
                        z7[:, yy, :, :, :].rearrange("p a x b -> p (a x b)")
                        .rearrange("p (a b) -> p a b", b=512),
                        ps[:], AF.Relu, bias=sb2[:, 0, 1:2],
                        scale=sb2[:, 0, 0:1])
                zx = pt.tile([128, 2, 2, 4, B], dt.float32, tag="zx")
                for yy in range(2):
                    nc.vector.tensor_tensor(
                        zx[:, yy, :, :, :], z7[:, yy, :, 0::2, :],
                        z7[:, yy, :, 1::2, :], OP.max)
                zp = pt.tile([128, 2, 4, B], dt.float32, tag="zp")
                nc.vector.tensor_tensor(zp[:], zx[:, 0, :, :, :],
                                        zx[:, 1, :, :, :], OP.max)
                zpf = zp[:].rearrange("p a x b -> p (a x b)")
                nc.vector.tensor_scalar(zpf, zpf, MAGIC, MAGIC + 7.0,
                                        OP.add, OP.min)
                a3t = pt.tile([128, 2, 4, B], FP8, tag="a3t")
                nc.scalar.activation(
                    a3t[:].rearrange("p a x b -> p (a x b)"), zpf,
                    AF.Copy, bias=-MAGIC)
                nc.sync.dma_start(A3[0:64, 1 + yo, 1:9, :],
                                  a3t[0:64].rearrange("p a x b -> p (a x) b"))
                nc.sync.dma_start(A3[0:64, 1 + yo, 9:17, :],
                                  a3t[64:128].rearrange(
                                      "p a x b -> p (a x) b"))
                # x+1 dup copy for L3 dx-packing, 4 chunks
                if yo in (2, 7, 12, 15):
                    r0, r1 = {2: (0, 4), 7: (4, 9), 12: (9, 14),
                              15: (14, 18)}[yo]
                    nc.sync.dma_start(A3[64:128, r0:r1, 0:17, :],
                                      A3[0:64, r0:r1, 1:18, :])

        nc.gpsimd.dma_start(wf1[:], wf1d[:])

        # ------------- Layer 3 (64 -> 128, 16x16, dx-packed) --------------
        pa4_cm = tc.tile_pool(name="A4", bufs=1)
        pa4 = pa4_cm.__enter__()
        A4 = pa4.tile([128, 18, 18, B], FP8, tag="A4")
        zero_border(A4, 18)
        with (tc.tile_pool(name="c3ps", bufs=3, space="PSUM") as pps,
              tc.tile_pool(name="c3z", bufs=3) as pz):
            w3, sb3 = wsb[3], sbt[3]
            for y in range(16):
                ps = pps.tile([128, 2, 512], dt.float32, tag="ps")
                for xh in range(2):
                    x0 = 8 * xh
                    for dy in range(3):
                        base = A3[:, y + dy, x0:x0 + 8, :]
                        nc.tensor.matmul(
                            ps[:, xh, :], w3[:, 2 * dy:2 * dy + 2, :],
                            with_pair(base, 2 * B), start=(dy == 0),
                            stop=(dy == 2), perf_mode=DR)
                z7 = pz.tile([128, 2, 8, B], dt.float32, tag="z")
                nc.scalar.activation(
                    z7[:].rearrange("p a x b -> p (a x b)")
                    .rearrange("p (a b) -> p a b", b=512),
                    ps[:], AF.Relu, bias=sb3[:, 0, 1:2], scale=sb3[:, 0, 0:1])
                zf = z7[:].rearrange("p a x b -> p (a x b)")
                nc.vector.tensor_scalar(zf, zf, MAGIC, MAGIC + 7.0,
                                        OP.add, OP.min)
                nc.scalar.activation(
                    A4[:, 1 + y, 1:9, :].rearrange("p x b -> p (x b)"),
                    z7[:, 0, :, :].rearrange("p x b -> p (x b)"),
                    AF.Copy, bias=-MAGIC)
                nc.vector.tensor_scalar(
                    A4[:, 1 + y, 9:17, :].rearrange("p x b -> p (x b)"),
                    z7[:, 1, :, :].rearrange("p x b -> p (x b)"),
                    MAGIC, None, OP.subtract)

        # ------------- Layer 4 (128 -> 128, 16x16, pool -> 8) -------------
        pa5_cm = tc.tile_pool(name="A5", bufs=1)
        pa5 = pa5_cm.__enter__()
        A5 = pa5.tile([128, 10, 10, B], FP8, tag="A5")
        zero_border(A5, 10)
        d4 = _pair_deltas(18)
        with (tc.tile_pool(name="c4ps", bufs=3, space="PSUM") as pps,
              tc.tile_pool(name="c4z", bufs=2) as pz,
              tc.tile_pool(name="c4t", bufs=2) as pt):
            w4, sb4 = wsb[4], sbt[4]
            for yo in range(8):
                z7 = pz.tile([128, 2, 2, 8, B], dt.float32, tag="z")
                for yy in range(2):
                    y = 2 * yo + yy
                    ps = pps.tile([128, 2, 512], dt.float32, tag="ps")
                    for xh in range(2):
                        x0 = 8 * xh

                        def mkbase(p, _y=y, _x0=x0):
                            dy, dx = TAPS[2 * p] if p < 4 else TAPS[8]
                            return A4[:, _y + dy, _x0 + dx:_x0 + dx + 8, :]
                        dr_group(ps[:, xh, :], w4, slice(0, 128), mkbase,
                                 d4, 5)
                    nc.scalar.activation(
                        z7[:, yy, :, :, :].rearrange("p a x b -> p (a x b)")
                        .rearrange("p (a b) -> p a b", b=512),
                        ps[:], AF.Relu, bias=sb4[:, 0, 1:2],
                        scale=sb4[:, 0, 0:1])
                zx = pt.tile([128, 2, 2, 4, B], dt.float32, tag="zx")
                for yy in range(2):
                    nc.vector.tensor_tensor(
                        zx[:, yy, :, :, :], z7[:, yy, :, 0::2, :],
                        z7[:, yy, :, 1::2, :], OP.max)
                zp = pt.tile([128, 2, 4, B], dt.float32, tag="zp")
                nc.vector.tensor_tensor(zp[:], zx[:, 0, :, :, :],
                                        zx[:, 1, :, :, :], OP.max)
                zpf = zp[:].rearrange("p a x b -> p (a x b)")
                nc.vector.tensor_scalar(zpf, zpf, MAGIC, MAGIC + 7.0,
                                        OP.add, OP.min)
                nc.scalar.activation(
                    A5[:, 1 + yo, 1:9, :].rearrange("p x b -> p (x b)"),
                    zpf, AF.Copy, bias=-MAGIC)

        # ------------- Layer 5 (128 -> 256, 8x8) --------------------------
        pa6_cm = tc.tile_pool(name="A6", bufs=1)
        pa6 = pa6_cm.__enter__()
        A6 = pa6.tile([128, 2, 10, 10, B], FP8, tag="A6")
        nc.gpsimd.memset(A6[:, :, 0, :, :], 0.0)
        nc.gpsimd.memset(A6[:, :, 9, :, :], 0.0)
        nc.gpsimd.memset(A6[:, :, 1:9, 0, :], 0.0)
        nc.gpsimd.memset(A6[:, :, 1:9, 9, :], 0.0)
        d5 = _pair_deltas(10)
        with (tc.tile_pool(name="c5ps", bufs=3, space="PSUM") as pps,
              tc.tile_pool(name="c5z", bufs=3) as pz):
            w5, sb5 = wsb[5], sbt[5]
            for y in range(8):
                ps = pps.tile([128, 2, 512], dt.float32, tag="ps")
                for ct in range(2):
                    def mkbase(p, _y=y):
                        dy, dx = TAPS[2 * p] if p < 4 else TAPS[8]
                        return A5[:, _y + dy, dx:dx + 8, :]
                    dr_group(ps[:, ct, :], w5,
                             slice(ct * 128, ct * 128 + 128), mkbase, d5, 5)
                z7 = pz.tile([128, 2, 8, B], dt.float32, tag="z")
                for ct in range(2):
                    nc.scalar.activation(
                        z7[:, ct, :, :].rearrange("p x b -> p (x b)"),
                        ps[:, ct, :],
                        AF.Relu, bias=sb5[:, ct, 1:2], scale=sb5[:, ct, 0:1])
                zf = z7[:].rearrange("p c x b -> p (c x b)")
                nc.vector.tensor_scalar(zf, zf, MAGIC, MAGIC + 7.0,
                                        OP.add, OP.min)
                nc.scalar.activation(
                    A6[:, 0, 1 + y, 1:9, :].rearrange("p x b -> p (x b)"),
                    z7[:, 0, :, :].rearrange("p x b -> p (x b)"),
                    AF.Copy, bias=-MAGIC)
                nc.vector.tensor_scalar(
                    A6[:, 1, 1 + y, 1:9, :].rearrange("p x b -> p (x b)"),
                    z7[:, 1, :, :].rearrange("p x b -> p (x b)"),
                    MAGIC, None, OP.subtract)

        # ------------- Layer 6 (256 -> 256, 8x8, pool -> 4) ---------------
        pa7_cm = tc.tile_pool(name="A7", bufs=1)
        pa7 = pa7_cm.__enter__()
        A7 = pa7.tile([128, 2, 4, 4, B], FP8, tag="A7")  # unpadded, feeds FC
        d6 = _pair_deltas(10)
        CIG = 10 * 10 * B  # element offset between the two ci-groups of A6
        with (tc.tile_pool(name="c6ps", bufs=3, space="PSUM") as pps,
              tc.tile_pool(name="c6z", bufs=2) as pz,
              tc.tile_pool(name="c6t", bufs=2) as pt):
            w6, sb6 = wsb[6], sbt[6]
            for yo in range(4):
                z7 = pz.tile([128, 2, 2, 8, B], dt.float32, tag="z")
                for yy in range(2):
                    y = 2 * yo + yy
                    ps = pps.tile([128, 2, 512], dt.float32, tag="ps")
                    for ct in range(2):
                        co_sl = slice(ct * 128, ct * 128 + 128)
                        for p in range(9):
                            if p < 8:
                                cig, pp = p // 4, p % 4
                                dy, dx = TAPS[2 * pp]
                                base = A6[:, cig, y + dy, dx:dx + 8, :]
                                delta = d6[pp]
                            else:
                                base = A6[:, 0, y + 2, 2:2 + 8, :]
                                delta = CIG
                            nc.tensor.matmul(
                                ps[:, ct, :], w6[:, p, :, co_sl],
                                with_pair(base, delta),
                                start=(p == 0), stop=(p == 8), perf_mode=DR)
                    for ct in range(2):
                        nc.scalar.activation(
                            z7[:, yy, ct, :, :].rearrange(
                                "p x b -> p (x b)"),
                            ps[:, ct, :],
                            AF.Relu, bias=sb6[:, ct, 1:2],
                            scale=sb6[:, ct, 0:1])
                zx = pt.tile([128, 2, 2, 4, B], dt.float32, tag="zx")
                for yy in range(2):
                    nc.vector.tensor_tensor(
                        zx[:, yy, :, :, :], z7[:, yy, :, 0::2, :],
                        z7[:, yy, :, 1::2, :], OP.max)
                zp = pt.tile([128, 2, 4, B], dt.float32, tag="zp")
                nc.vector.tensor_tensor(zp[:], zx[:, 0, :, :, :],
                                        zx[:, 1, :, :, :], OP.max)
                zpf = zp[:].rearrange("p c x b -> p (c x b)")
                nc.vector.tensor_scalar(zpf, zpf, MAGIC, MAGIC + 7.0,
                                        OP.add, OP.min)
                nc.scalar.activation(A7[:, :, yo, :, :], zp[:],
                                     AF.Copy, bias=-MAGIC)

        # ------------- FC1 (4096 -> 512) ----------------------------------
        pa8_cm = tc.tile_pool(name="A8", bufs=1)
        pa8 = pa8_cm.__enter__()
        A8 = pa8.tile([128, 4, B], FP8, tag="A8")
        with (tc.tile_pool(name="f1ps", bufs=4, space="PSUM") as pps,
              tc.tile_pool(name="f1t", bufs=4) as pt):
            for ct in range(4):
                ps = pps.tile([128, B], dt.float32, tag="ps")
                k = 0
                for cig in range(2):
                    for px in range(16):
                        wo = (cig * 16 + px) * 512 + ct * 128
                        nc.tensor.matmul(ps[:], wf1[:, wo:wo + 128],
                                         A7[:, cig, px // 4, px % 4, :],
                                         start=(k == 0), stop=(k == 31))
                        k += 1
                z7 = pt.tile([128, B], dt.float32, tag="z")
                nc.scalar.activation(z7[:], ps[:], AF.Relu,
                                     bias=sbf1[:, ct, 1:2],
                                     scale=sbf1[:, ct, 0:1])
                nc.vector.tensor_scalar(z7[:], z7[:], MAGIC, MAGIC + 7.0,
                                        OP.add, OP.min)
                nc.scalar.activation(A8[:, ct, :], z7[:], AF.Copy,
                                      bias=-MAGIC)

        # ------------- FC2 (512 -> 10), signed output ---------------------
        with (tc.tile_pool(name="f2ps", bufs=1, space="PSUM") as pps,
              tc.tile_pool(name="f2t", bufs=1) as pt):
            ps = pps.tile([10, B], dt.float32, tag="ps")
            for kt in range(4):
                nc.tensor.matmul(ps[:], wf2[:, kt * 10:(kt + 1) * 10],
                                 A8[:, kt, :], start=(kt == 0), stop=(kt == 3))
            z7 = pt.tile([10, B], dt.float32, tag="z")
            nc.vector.tensor_scalar(z7[:], ps[:], sbf2[:, 0:1], sbf2[:, 1:2],
                                    OP.mult, OP.add)
            r = pt.tile([10, B], dt.float32, tag="r")
            nc.vector.tensor_scalar(r[:], z7[:], MAGIC, MAGIC - 7.0,
                                    OP.add, OP.max)  # RNE + lower clamp
            r2 = pt.tile([10, B], dt.float32, tag="r2")
            nc.vector.tensor_scalar(r2[:], r[:], MAGIC + 7.0, MAGIC,
                                    OP.min, OP.subtract)
            fin = pt.tile([10, B], dt.float32, tag="fin")
            nc.vector.tensor_scalar(fin[:], r2[:], 1.0 / 7.0,
                                    None, OP.mult)
            nc.sync.dma_start(outd[:].rearrange("b c -> c b"), fin[:])
        for cm in (pa8_cm, pa7_cm, pa6_cm, pa5_cm, pa4_cm, pa3_cm, fcw_cm,
                   pa2_cm):
            cm.__exit__(None, None, None)
        wp_cm.__exit__(None, None, None)

    nc.compile()
    return nc


# ----------------------------------------------------------------------------
# Entry point
# ----------------------------------------------------------------------------

_NC_CACHE = {}
LAST_RESULTS = None  # BassKernelResults of the most recent run (for test.py)


def kernel(**inputs):
    global LAST_RESULTS
    from concourse.bass_utils import run_bass_kernel_spmd
    if "nc" not in _NC_CACHE:
        _NC_CACHE["nc"] = build_nc()
    nc = _NC_CACHE["nc"]
    in_maps = host_pack(inputs)
    res = run_bass_kernel_spmd(nc, in_maps, list(range(N_CORES)))
    LAST_RESULTS = res
    outs = [res.results[c]["out"] for c in range(N_CORES)]
    return np.concatenate(outs, axis=0).astype(np.float32)


# revision 12
# speedup vs baseline: 3.5726x; 1.0201x over previous
"""Trainium2 Bass kernel for nn_IntegerCifar10Net (quantized VGG-ish CNN).

Data parallel over 8 NeuronCores, B=64 images/core.

v3: engine-balanced quant chain + upfront chunked xcol streaming.

Layer matmul schemes (unchanged from v2 except L6):
  L1 : exact 3-plane bf16 im2col (K=81), co=64; x-halves to PSUM partitions
       0-63 / 64-127; quantized row writes A2 lower+upper halves in one op.
  L2 : "halves" trick - PSUM partitions 0-63 = left 16 output cols,
       64-127 = right 16; block-diagonal weights, 5 DR matmuls per bank.
  L3 : dx-packing (x+1 dup in upper partitions): 3 DR matmuls per bank.
  L4/L5: plain 9 taps -> 5 DR matmuls per bank.
  L6 : 9 DR matmuls per bank - the two odd 9th taps of the two ci-groups
       share one DR pair (cig-pairing) instead of 2 zero-padded pairs.

Quant chain per bank, balanced across Scalar(Act) and Vector(DVE) (the
GpSimd/Pool engine only has slow Q7-ucode elementwise ops on TRN2, and
they also stall DVE via the shared SBUF port - measured 10.6us/op):
  Act : z = relu(psum*scale + bias)                  [PSUM -> SBUF f32]
  DVE : u = min(z + MAGIC, MAGIC+7)  (in-place)      [RNE round + clamp]
  Act/DVE : a = u - MAGIC -> fp8 (Copy activation with immediate bias
  -MAGIC on Act, tensor_scalar on DVE; split to balance engine load)
Max-pool layers run the pairwise maxes on DVE over pre-round z.

xcol is staged fully in SBUF via 8 upfront chunk DMAs (4 rows each)
round-robined over the sync/scalar/gpsimd queues; weights stream behind.
"""

import sys
import numpy as np

sys.path.insert(0, "/opt/trn_rl_repo")

import ml_dtypes

N_CORES = 8
B = 64  # images per core
MAGIC = 12582912.0  # 1.5 * 2^23 : RNE rounding magic for |v| < 2^22
N_PLANES = 3  # bf16 planes for exact L1 input (hi/mid/lo)
K1 = 27 * N_PLANES

# tap pair schedule for 3x3 convs: pairs of taps t=(dy,dx) row-major,
# 10th tap is zero-weight padding with moving delta -B (always in bounds)
TAPS = [(dy, dx) for dy in range(3) for dx in range(3)]


def _pair_deltas(W):
    """Moving-AP element deltas between the two taps of each DR pair."""
    ds = []
    for p in range(4):
        (dya, dxa), (dyb, dxb) = TAPS[2 * p], TAPS[2 * p + 1]
        ds.append(((dyb - dya) * W + (dxb - dxa)) * B)
    ds.append(-B)  # pad pair: (t8, zero-weight tap at x-1)
    return ds


# ----------------------------------------------------------------------------
# Host-side packing
# ----------------------------------------------------------------------------

def _qint(w):
    """round(clip(w,-1,1)*7) as float32 integers, matching jax fp32 chain."""
    w = np.asarray(w, np.float32)
    return np.round(np.clip(w, -1.0, 1.0) * np.float32(7.0)).astype(np.float32)


def _scale_bias(g, b, denom):
    # z7 = conv_int * (7*g/denom) + 7*b, constants in f64 then rounded to f32
    s = (7.0 * np.asarray(g, np.float64) / denom).astype(np.float32)
    bt = (7.0 * np.asarray(b, np.float64)).astype(np.float32)
    return np.ascontiguousarray(np.stack([s, bt], axis=1))  # [co, 2] f32


def _im2col_bf16(x):
    """x [B,3,32,32] f32 -> [K1, 32, 32, B] bf16 (N_PLANES x 27 rows)."""
    Bc = x.shape[0]
    xp = np.zeros((Bc, 3, 34, 34), np.float32)
    xp[:, :, 1:33, 1:33] = x
    planes = np.empty((27, 32, 32, Bc), np.float32)
    k = 0
    for ci in range(3):
        for dy in range(3):
            for dx in range(3):
                planes[k] = np.transpose(xp[:, ci, dy:dy + 32, dx:dx + 32],
                                         (1, 2, 0))
                k += 1
    out = []
    rem = planes
    for _ in range(N_PLANES):
        p = rem.astype(ml_dtypes.bfloat16)
        out.append(p)
        rem = rem - p.astype(np.float32)
    return np.ascontiguousarray(np.concatenate(out, axis=0))


def host_pack(inputs):
    """Build the per-core DRAM input dicts (weights replicated)."""
    f8 = ml_dtypes.float8_e4m3
    wc = {}
    # L1 weights: [64,3,3,3] -> lhsT [27,64], replicated per plane
    t = np.transpose(_qint(inputs["w1"]), (1, 2, 3, 0)).reshape(27, 64)
    wc["w1sb"] = np.ascontiguousarray(
        np.concatenate([t] * N_PLANES, axis=0).astype(ml_dtypes.bfloat16))
    sb = _scale_bias(inputs["g1"], inputs["b1"], 7.0)  # [64, 2]
    wc["sb1"] = np.ascontiguousarray(
        np.concatenate([sb, sb], axis=0).reshape(128, 1, 2))
    # L2: halves block-diagonal [128, 10, 128]
    wq = _qint(inputs["w2"])  # [64co, 64ci, 3, 3]
    w2p = np.zeros((128, 10, 128), np.float32)
    for ti, (dy, dx) in enumerate(TAPS):
        blk = wq[:, :, dy, dx].T  # [ci, co]
        w2p[0:64, ti, 0:64] = blk
        w2p[64:128, ti, 64:128] = blk
    wc["w2sb"] = np.ascontiguousarray(w2p.astype(f8))
    sb = _scale_bias(inputs["g2"], inputs["b2"], 49.0)
    wc["sb2"] = np.ascontiguousarray(
        np.concatenate([sb, sb], axis=0).reshape(128, 1, 2))
    # L3: dx-packed virtual taps [128, 6, 128]
    wq = _qint(inputs["w3"])  # [128co, 64ci, 3, 3]
    w3p = np.zeros((128, 6, 128), np.float32)
    for dy in range(3):
        for oi, o in enumerate((0, 2)):
            v = 2 * dy + oi
            w3p[0:64, v, :] = wq[:, :, dy, o].T
            if o == 0:
                w3p[64:128, v, :] = wq[:, :, dy, 1].T
    wc["w3sb"] = np.ascontiguousarray(w3p.astype(f8))
    wc["sb3"] = np.ascontiguousarray(
        _scale_bias(inputs["g3"], inputs["b3"], 49.0).reshape(128, 1, 2))
    # L4: [128, 10, 128]
    wq = _qint(inputs["w4"])  # [128, 128, 3, 3]
    w4p = np.zeros((128, 10, 128), np.float32)
    for ti, (dy, dx) in enumerate(TAPS):
        w4p[:, ti, :] = wq[:, :, dy, dx].T
    wc["w4sb"] = np.ascontiguousarray(w4p.astype(f8))
    wc["sb4"] = np.ascontiguousarray(
        _scale_bias(inputs["g4"], inputs["b4"], 49.0).reshape(128, 1, 2))
    # L5: [128, 10, 256]
    wq = _qint(inputs["w5"])  # [256, 128, 3, 3]
    w5p = np.zeros((128, 10, 256), np.float32)
    for ti, (dy, dx) in enumerate(TAPS):
        w5p[:, ti, :] = wq[:, :, dy, dx].T
    wc["w5sb"] = np.ascontiguousarray(w5p.astype(f8))
    wc["sb5"] = np.ascontiguousarray(
        _scale_bias(inputs["g5"], inputs["b5"], 49.0).reshape(2, 128, 2)
        .transpose(1, 0, 2))
    # L6: [128, 9, 2, 256] cig-paired: slots 0-3 pair taps (2p,2p+1) of
    # cig0, slots 4-7 the same of cig1, slot 8 pairs (t8@cig0, t8@cig1).
    wq = _qint(inputs["w6"])  # [256, 256, 3, 3]
    w6p = np.zeros((128, 9, 2, 256), np.float32)
    for cig in range(2):
        for p in range(4):
            (dya, dxa), (dyb, dxb) = TAPS[2 * p], TAPS[2 * p + 1]
            w6p[:, 4 * cig + p, 0, :] = wq[:, cig * 128:(cig + 1) * 128,
                                           dya, dxa].T
            w6p[:, 4 * cig + p, 1, :] = wq[:, cig * 128:(cig + 1) * 128,
                                           dyb, dxb].T
    w6p[:, 8, 0, :] = wq[:, 0:128, 2, 2].T
    w6p[:, 8, 1, :] = wq[:, 128:256, 2, 2].T
    wc["w6sb"] = np.ascontiguousarray(w6p.astype(f8))
    wc["sb6"] = np.ascontiguousarray(
        _scale_bias(inputs["g6"], inputs["b6"], 49.0).reshape(2, 128, 2)
        .transpose(1, 0, 2))
    # FC1 [512, 4096]: k=(c,y,x), c=cig*128+p  -> [128, (cig,16,512)]
    t = _qint(inputs["wf1"]).T.reshape(2, 128, 16, 512)
    wc["wf1sb"] = np.ascontiguousarray(
        np.transpose(t, (1, 0, 2, 3)).reshape(128, 2 * 16 * 512).astype(f8))
    wc["sbf1"] = np.ascontiguousarray(_scale_bias(
        inputs["gf1"], inputs["bf1"], 49.0).reshape(4, 128, 2).transpose(
        1, 0, 2))
    # FC2 [10, 512] -> [128, (4,10)]
    t = _qint(inputs["wf2"]).T.reshape(4, 128, 10)
    wc["wf2sb"] = np.ascontiguousarray(
        np.transpose(t, (1, 0, 2)).reshape(128, 40).astype(f8))
    wc["sbf2"] = _scale_bias(inputs["gf2"], inputs["bf2"], 49.0)

    x = np.asarray(inputs["x"], np.float32)
    maps = []
    for c in range(N_CORES):
        m = dict(wc)
        m["xcol"] = _im2col_bf16(x[c * B:(c + 1) * B])
        maps.append(m)
    return maps


# ----------------------------------------------------------------------------
# Bass program
# ----------------------------------------------------------------------------

def build_nc():
    import concourse.bacc as bacc
    import concourse.mybir as mybir
    import concourse.tile as tile

    dt = mybir.dt
    AF = mybir.ActivationFunctionType
    OP = mybir.AluOpType
    FP8 = dt.float8e4
    DR = mybir.MatmulPerfMode.DoubleRow

    nc = bacc.Bacc("TRN2", target_bir_lowering=False, debug=False)

    xcold = nc.dram_tensor("xcol", [K1, 32, 32, B], dt.bfloat16,
                           kind="ExternalInput")
    w1d = nc.dram_tensor("w1sb", [K1, 64], dt.bfloat16, kind="ExternalInput")
    w2d = nc.dram_tensor("w2sb", [128, 10, 128], FP8, kind="ExternalInput")
    w3d = nc.dram_tensor("w3sb", [128, 6, 128], FP8, kind="ExternalInput")
    w4d = nc.dram_tensor("w4sb", [128, 10, 128], FP8, kind="ExternalInput")
    w5d = nc.dram_tensor("w5sb", [128, 10, 256], FP8, kind="ExternalInput")
    w6d = nc.dram_tensor("w6sb", [128, 9, 2, 256], FP8, kind="ExternalInput")
    sbd = {}
    sbshape = {1: [128, 1, 2], 2: [128, 1, 2], 3: [128, 1, 2],
               4: [128, 1, 2], 5: [128, 2, 2], 6: [128, 2, 2]}
    for i in range(1, 7):
        sbd[i] = nc.dram_tensor(f"sb{i}", sbshape[i], dt.float32,
                                kind="ExternalInput")
    wf1d = nc.dram_tensor("wf1sb", [128, 2 * 16 * 512], FP8,
                          kind="ExternalInput")
    sbf1d = nc.dram_tensor("sbf1", [128, 4, 2], dt.float32,
                           kind="ExternalInput")
    wf2d = nc.dram_tensor("wf2sb", [128, 40], FP8, kind="ExternalInput")
    sbf2d = nc.dram_tensor("sbf2", [10, 2], dt.float32, kind="ExternalInput")
    outd = nc.dram_tensor("out", [B, 10], dt.float32, kind="ExternalOutput")

    with tile.TileContext(nc) as tc:
        # ------------- persistent weights (tiles only, DMAs below) --------
        wp_cm = tc.tile_pool(name="weights", bufs=1)
        wp = wp_cm.__enter__()
        w1 = wp.tile([K1, 64], dt.bfloat16, tag="w1")
        wsb = {}
        for i, shape in ((2, [128, 10, 128]), (3, [128, 6, 128]),
                         (4, [128, 10, 128]), (5, [128, 10, 256]),
                         (6, [128, 9, 2, 256])):
            t = wp.tile(shape, FP8, tag=f"w{i}")
            wsb[i] = t
        sbt = {}
        sbt0 = {}
        for i in range(1, 7):
            t0 = wp.tile(sbshape[i], dt.float32, tag=f"s{i}d")
            sbt0[i] = t0
            t = wp.tile(sbshape[i], dt.float32, tag=f"s{i}")
            sbt[i] = t
        wf2 = wp.tile([128, 40], FP8, tag="wf2")
        sbf2t = wp.tile([10, 2], dt.float32, tag="sf2d")
        sbf2 = wp.tile([10, 2], dt.float32, tag="sf2")

        def zero_border(A, Hp, eng=None):
            e = eng or nc.gpsimd
            e.memset(A[:, 0, :, :], 0.0)
            e.memset(A[:, Hp - 1, :, :], 0.0)
            e.memset(A[:, 1:Hp - 1, 0, :], 0.0)
            e.memset(A[:, 1:Hp - 1, Hp - 1, :], 0.0)

        def with_pair(ap, delta):
            APc = type(ap)
            pairs = list(ap.ap)
            return APc(ap.tensor, ap.offset,
                       [pairs[0], [delta, 2]] + list(pairs[1:]))

        def dr_group(ps_ap, wt, co_sl, base_fn, deltas, npairs, extra=None):
            """Accumulate npairs DR matmuls (+ optional extra groups)."""
            for p in range(npairs):
                nc.tensor.matmul(ps_ap, wt[:, 2 * p:2 * p + 2, co_sl],
                                 with_pair(base_fn(p), deltas[p]),
                                 start=(p == 0), stop=(extra is None
                                                       and p == npairs - 1),
                                 perf_mode=DR)
            if extra is not None:
                wt2, base_fn2, deltas2 = extra
                for p in range(npairs):
                    nc.tensor.matmul(ps_ap, wt2[:, 2 * p:2 * p + 2, co_sl],
                                     with_pair(base_fn2(p), deltas2[p]),
                                     start=False, stop=(p == npairs - 1),
                                     perf_mode=DR)

        # ------------- Layer 1: K=81 im2col conv, x-halves in PSUM --------
        # PSUM partitions 0-63 = left 16 output cols, 64-127 = right 16.
        # The fp8 write then fills A2's lower (channels, x) AND upper
        # (x+16 view) halves in one 128-lane op; only two boundary columns
        # (upper col0 = real x15, lower col17 = real x16) need patch DMAs.
        pa2_cm = tc.tile_pool(name="A2", bufs=1)
        pa2 = pa2_cm.__enter__()
        A2 = pa2.tile([128, 34, 18, B], FP8, tag="A2")

        # xcol staged fully in SBUF: 8 upfront chunk DMAs, 3 queues
        pxc_cm = tc.tile_pool(name="l1mov", bufs=8)
        pxc = pxc_cm.__enter__()
        chunks = []
        chunk_engs = (nc.sync, nc.scalar, nc.gpsimd)
        for c in range(8):
            ck = pxc.tile([K1, 4, 32, B], dt.bfloat16, tag="chunk")
            chunk_engs[c % 3].dma_start(ck[:], xcold[:, 4 * c:4 * c + 4, :, :])
            chunks.append(ck)
        nc.sync.dma_start(w1[:], w1d[:])
        nc.scalar.dma_start(wsb[2][:], w2d[:])
        nc.sync.dma_start(sbt0[1][:], sbd[1][:])
        nc.scalar.copy(sbt[1][:], sbt0[1][:])
        nc.vector.memset(A2[:, 0, :, :], 0.0)
        nc.vector.memset(A2[:, 33, :, :], 0.0)
        nc.gpsimd.memset(A2[:, 1:33, 0, :], 0.0)
        nc.gpsimd.memset(A2[:, 1:33, 17, :], 0.0)
        def l1_store(q, z7):
            # fp8 store for q's two rows, one on each engine (issued one
            # iteration late so neither engine head-of-line blocks)
            nc.vector.tensor_scalar(
                A2[:, 1 + 2 * q, 1:17, :].rearrange("p x b -> p (x b)"),
                z7[:, 0:2, :].rearrange("p g b -> p (g b)"),
                MAGIC, None, OP.subtract)
            nc.scalar.activation(
                A2[:, 2 + 2 * q, 1:17, :].rearrange("p x b -> p (x b)"),
                z7[:, 2:4, :].rearrange("p g b -> p (g b)"),
                AF.Copy, bias=-MAGIC)

        with (tc.tile_pool(name="l1ps", bufs=2, space="PSUM") as pps,
              tc.tile_pool(name="l1z", bufs=3) as pz):
            prev = None
            for q in range(16):  # y-pairs
                ck = chunks[q // 2]
                ps = pps.tile([128, 4, 512], dt.float32, tag="ps")
                for yy in range(2):
                    yc = 2 * (q % 2) + yy
                    for g in range(2):
                        nc.tensor.matmul(ps[0:64, 2 * yy + g, :], w1[:],
                                         ck[:, yc, g * 8:g * 8 + 8, :],
                                         start=True, stop=True)
                        nc.tensor.matmul(ps[64:128, 2 * yy + g, :], w1[:],
                                         ck[:, yc, 16 + g * 8:
                                            16 + g * 8 + 8, :],
                                         start=True, stop=True)
                z7 = pz.tile([128, 4, 512], dt.float32, tag="z")
                nc.scalar.activation(z7[:], ps[:], AF.Relu,
                                     bias=sbt[1][:, 0, 1:2],
                                     scale=sbt[1][:, 0, 0:1])
                zf = z7[:].rearrange("p a b -> p (a b)")
                nc.vector.tensor_scalar(zf, zf, MAGIC, MAGIC + 7.0,
                                        OP.add, OP.min)
                if prev is not None:
                    l1_store(q - 1, prev)
                prev = z7
            l1_store(15, prev)
        # boundary patch columns (after all xcol chunks: keep queues clean)
        for k in range(4):
            r0 = 1 + 8 * k
            nc.gpsimd.dma_start(A2[64:128, r0:r0 + 8, 0:1, :],
                                A2[0:64, r0:r0 + 8, 16:17, :])
            nc.gpsimd.dma_start(A2[0:64, r0:r0 + 8, 17:18, :],
                                A2[64:128, r0:r0 + 8, 1:2, :])
        # stream the remaining weights behind the L1 loads
        nc.scalar.dma_start(wsb[3][:], w3d[:])
        nc.scalar.dma_start(wsb[4][:], w4d[:])
        nc.sync.dma_start(wsb[5][:], w5d[:])
        nc.sync.dma_start(wsb[6][:], w6d[:])
        for i in range(2, 7):
            nc.sync.dma_start(sbt0[i][:], sbd[i][:])
            nc.scalar.copy(sbt[i][:], sbt0[i][:])
        nc.sync.dma_start(wf2[:], wf2d[:])
        nc.sync.dma_start(sbf2t[:], sbf2d[:])
        nc.scalar.copy(sbf2[:], sbf2t[:])
        pxc_cm.__exit__(None, None, None)  # free the xcol staging space
        fcw_cm = tc.tile_pool(name="fcw", bufs=1)
        fcw = fcw_cm.__enter__()
        wf1 = fcw.tile([128, 2 * 16 * 512], FP8, tag="wf1")
        sbf1t = fcw.tile([128, 4, 2], dt.float32, tag="sf1d")
        sbf1 = fcw.tile([128, 4, 2], dt.float32, tag="sf1")
        nc.scalar.dma_start(sbf1t[:], sbf1d[:])
        nc.scalar.copy(sbf1[:], sbf1t[:])

        # ------------- Layer 2 (64ch 32x32, halves, pool -> 16) -----------
        pa3_cm = tc.tile_pool(name="A3", bufs=1)
        pa3 = pa3_cm.__enter__()
        A3 = pa3.tile([128, 18, 18, B], FP8, tag="A3")
        zero_border(A3, 18)
        d2 = _pair_deltas(18)
        with (tc.tile_pool(name="c2ps", bufs=3, space="PSUM") as pps,
              tc.tile_pool(name="c2z", bufs=2) as pz,
              tc.tile_pool(name="c2t", bufs=2) as pt):
            w2, sb2 = wsb[2], sbt[2]

            def l2_store(yo, zp):
                a3t = pt.tile([128, 2, 4, B], FP8, tag="a3t")
                nc.scalar.activation(
                    a3t[:].rearrange("p a x b -> p (a x b)"),
                    zp[:].rearrange("p a x b -> p (a x b)"),
                    AF.Copy, bias=-MAGIC)
                nc.sync.dma_start(A3[0:64, 1 + yo, 1:9, :],
                                  a3t[0:64].rearrange("p a x b -> p (a x) b"))
                nc.sync.dma_start(A3[0:64, 1 + yo, 9:17, :],
                                  a3t[64:128].rearrange(
                                      "p a x b -> p (a x) b"))

            prev = None
            for yo in range(16):
                z7 = pz.tile([128, 2, 2, 8, B], dt.float32, tag="z")
                for yy in range(2):
                    y = 2 * yo + yy
                    ps = pps.tile([128, 2, 512], dt.float32, tag="ps")
                    for xh in range(2):
                        x0 = 8 * xh

                        def mkbase(p, _y=y, _x0=x0):
                            dy, dx = TAPS[2 * p] if p < 4 else TAPS[8]
                            return A2[:, _y + dy, _x0 + dx:_x0 + dx + 8, :]
                        dr_group(ps[:, xh, :], w2, slice(0, 128), mkbase,
                                 d2, 5)
                    nc.scalar.activation(
                        z7[:, yy, :, :, :].rearrange("p a x b -> p (a x b)")
                        .rearrange("p (a b) -> p a b", b=512),
                        ps[:], AF.Relu, bias=sb2[:, 0, 1:2],
                        scale=sb2[:, 0, 0:1])
                zy = pt.tile([128, 2, 8, B], dt.float32, tag="zy")
                nc.vector.tensor_tensor(zy[:], z7[:, 0, :, :, :],
                                        z7[:, 1, :, :, :], OP.max)
                zp = pt.tile([128, 2, 4, B], dt.float32, tag="zp")
                nc.vector.tensor_tensor(zp[:], zy[:, :, 0::2, :],
                                        zy[:, :, 1::2, :], OP.max)
                zpf = zp[:].rearrange("p a x b -> p (a x b)")
                nc.vector.tensor_scalar(zpf, zpf, MAGIC, MAGIC + 7.0,
                                        OP.add, OP.min)
                if prev is not None:
                    l2_store(yo - 1, prev)
                prev = zp
                # x+1 dup copy for L3 dx-packing, 4 chunks (rows lag 1)
                if yo in (3, 8, 13):
                    r0, r1 = {3: (0, 4), 8: (4, 9), 13: (9, 14)}[yo]
                    nc.sync.dma_start(A3[64:128, r0:r1, 0:17, :],
                                      A3[0:64, r0:r1, 1:18, :])
            l2_store(15, prev)
            nc.sync.dma_start(A3[64:128, 14:18, 0:17, :],
                              A3[0:64, 14:18, 1:18, :])

        nc.gpsimd.dma_start(wf1[:], wf1d[:])

        # ------------- Layer 3 (64 -> 128, 16x16, dx-packed) --------------
        pa4_cm = tc.tile_pool(name="A4", bufs=1)
        pa4 = pa4_cm.__enter__()
        A4 = pa4.tile([128, 18, 18, B], FP8, tag="A4")
        zero_border(A4, 18)
        with (tc.tile_pool(name="c3ps", bufs=3, space="PSUM") as pps,
              tc.tile_pool(name="c3z", bufs=3) as pz):
            w3, sb3 = wsb[3], sbt[3]

            def l3_store(y, z7):
                nc.scalar.activation(
                    A4[:, 1 + y, 1:9, :].rearrange("p x b -> p (x b)"),
                    z7[:, 0, :, :].rearrange("p x b -> p (x b)"),
                    AF.Copy, bias=-MAGIC)
                nc.vector.tensor_scalar(
                    A4[:, 1 + y, 9:17, :].rearrange("p x b -> p (x b)"),
                    z7[:, 1, :, :].rearrange("p x b -> p (x b)"),
                    MAGIC, None, OP.subtract)

            prev = None
            for y in range(16):
                ps = pps.tile([128, 2, 512], dt.float32, tag="ps")
                for xh in range(2):
                    x0 = 8 * xh
                    for dy in range(3):
                        base = A3[:, y + dy, x0:x0 + 8, :]
                        nc.tensor.matmul(
                            ps[:, xh, :], w3[:, 2 * dy:2 * dy + 2, :],
                            with_pair(base, 2 * B), start=(dy == 0),
                            stop=(dy == 2), perf_mode=DR)
                z7 = pz.tile([128, 2, 8, B], dt.float32, tag="z")
                nc.scalar.activation(
                    z7[:].rearrange("p a x b -> p (a x b)")
                    .rearrange("p (a b) -> p a b", b=512),
                    ps[:], AF.Relu, bias=sb3[:, 0, 1:2], scale=sb3[:, 0, 0:1])
                zf = z7[:].rearrange("p a x b -> p (a x b)")
                nc.vector.tensor_scalar(zf, zf, MAGIC, MAGIC + 7.0,
                                        OP.add, OP.min)
                if prev is not None:
                    l3_store(y - 1, prev)
                prev = z7
            l3_store(15, prev)

        # ------------- Layer 4 (128 -> 128, 16x16, pool -> 8) -------------
        pa5_cm = tc.tile_pool(name="A5", bufs=1)
        pa5 = pa5_cm.__enter__()
        A5 = pa5.tile([128, 10, 10, B], FP8, tag="A5")
        zero_border(A5, 10)
        d4 = _pair_deltas(18)
        with (tc.tile_pool(name="c4ps", bufs=3, space="PSUM") as pps,
              tc.tile_pool(name="c4z", bufs=2) as pz,
              tc.tile_pool(name="c4t", bufs=2) as pt):
            w4, sb4 = wsb[4], sbt[4]
            prev = None
            for yo in range(8):
                z7 = pz.tile([128, 2, 2, 8, B], dt.float32, tag="z")
                for yy in range(2):
                    y = 2 * yo + yy
                    ps = pps.tile([128, 2, 512], dt.float32, tag="ps")
                    for xh in range(2):
                        x0 = 8 * xh

                        def mkbase(p, _y=y, _x0=x0):
                            dy, dx = TAPS[2 * p] if p < 4 else TAPS[8]
                            return A4[:, _y + dy, _x0 + dx:_x0 + dx + 8, :]
                        dr_group(ps[:, xh, :], w4, slice(0, 128), mkbase,
                                 d4, 5)
                    nc.scalar.activation(
                        z7[:, yy, :, :, :].rearrange("p a x b -> p (a x b)")
                        .rearrange("p (a b) -> p a b", b=512),
                        ps[:], AF.Relu, bias=sb4[:, 0, 1:2],
                        scale=sb4[:, 0, 0:1])
                zy = pt.tile([128, 2, 8, B], dt.float32, tag="zy")
                nc.vector.tensor_tensor(zy[:], z7[:, 0, :, :, :],
                                        z7[:, 1, :, :, :], OP.max)
                zp = pt.tile([128, 2, 4, B], dt.float32, tag="zp")
                nc.vector.tensor_tensor(zp[:], zy[:, :, 0::2, :],
                                        zy[:, :, 1::2, :], OP.max)
                zpf = zp[:].rearrange("p a x b -> p (a x b)")
                nc.vector.tensor_scalar(zpf, zpf, MAGIC, MAGIC + 7.0,
                                        OP.add, OP.min)
                if prev is not None:
                    nc.scalar.activation(
                        A5[:, yo, 1:9, :].rearrange("p x b -> p (x b)"),
                        prev[:].rearrange("p a x b -> p (a x b)"),
                        AF.Copy, bias=-MAGIC)
                prev = zp
            nc.scalar.activation(
                A5[:, 8, 1:9, :].rearrange("p x b -> p (x b)"),
                prev[:].rearrange("p a x b -> p (a x b)"),
                AF.Copy, bias=-MAGIC)

        # ------------- Layer 5 (128 -> 256, 8x8) --------------------------
        pa6_cm = tc.tile_pool(name="A6", bufs=1)
        pa6 = pa6_cm.__enter__()
        A6 = pa6.tile([128, 2, 10, 10, B], FP8, tag="A6")
        nc.gpsimd.memset(A6[:, :, 0, :, :], 0.0)
        nc.gpsimd.memset(A6[:, :, 9, :, :], 0.0)
        nc.gpsimd.memset(A6[:, :, 1:9, 0, :], 0.0)
        nc.gpsimd.memset(A6[:, :, 1:9, 9, :], 0.0)
        d5 = _pair_deltas(10)
        with (tc.tile_pool(name="c5ps", bufs=3, space="PSUM") as pps,
              tc.tile_pool(name="c5z", bufs=3) as pz):
            w5, sb5 = wsb[5], sbt[5]

            def l5_store(y, z7):
                nc.scalar.activation(
                    A6[:, 0, 1 + y, 1:9, :].rearrange("p x b -> p (x b)"),
                    z7[:, 0, :, :].rearrange("p x b -> p (x b)"),
                    AF.Copy, bias=-MAGIC)
                nc.vector.tensor_scalar(
                    A6[:, 1, 1 + y, 1:9, :].rearrange("p x b -> p (x b)"),
                    z7[:, 1, :, :].rearrange("p x b -> p (x b)"),
                    MAGIC, None, OP.subtract)

            prev = None
            for y in range(8):
                ps = pps.tile([128, 2, 512], dt.float32, tag="ps")
                for ct in range(2):
                    def mkbase(p, _y=y):
                        dy, dx = TAPS[2 * p] if p < 4 else TAPS[8]
                        return A5[:, _y + dy, dx:dx + 8, :]
                    dr_group(ps[:, ct, :], w5,
                             slice(ct * 128, ct * 128 + 128), mkbase, d5, 5)
                z7 = pz.tile([128, 2, 8, B], dt.float32, tag="z")
                for ct in range(2):
                    nc.scalar.activation(
                        z7[:, ct, :, :].rearrange("p x b -> p (x b)"),
                        ps[:, ct, :],
                        AF.Relu, bias=sb5[:, ct, 1:2], scale=sb5[:, ct, 0:1])
                zf = z7[:].rearrange("p c x b -> p (c x b)")
                nc.vector.tensor_scalar(zf, zf, MAGIC, MAGIC + 7.0,
                                        OP.add, OP.min)
                if prev is not None:
                    l5_store(y - 1, prev)
                prev = z7
            l5_store(7, prev)

        # ------------- Layer 6 (256 -> 256, 8x8, pool -> 4) ---------------
        pa7_cm = tc.tile_pool(name="A7", bufs=1)
        pa7 = pa7_cm.__enter__()
        A7 = pa7.tile([128, 2, 4, 4, B], FP8, tag="A7")  # unpadded, feeds FC
        d6 = _pair_deltas(10)
        CIG = 10 * 10 * B  # element offset between the two ci-groups of A6
        with (tc.tile_pool(name="c6ps", bufs=3, space="PSUM") as pps,
              tc.tile_pool(name="c6z", bufs=2) as pz,
              tc.tile_pool(name="c6t", bufs=2) as pt):
            w6, sb6 = wsb[6], sbt[6]
            for yo in range(4):
                z7 = pz.tile([128, 2, 2, 8, B], dt.float32, tag="z")
                for yy in range(2):
                    y = 2 * yo + yy
                    ps = pps.tile([128, 2, 512], dt.float32, tag="ps")
                    for ct in range(2):
                        co_sl = slice(ct * 128, ct * 128 + 128)
                        for p in range(9):
                            if p < 8:
                                cig, pp = p // 4, p % 4
                                dy, dx = TAPS[2 * pp]
                                base = A6[:, cig, y + dy, dx:dx + 8, :]
                                delta = d6[pp]
                            else:
                                base = A6[:, 0, y + 2, 2:2 + 8, :]
                                delta = CIG
                            nc.tensor.matmul(
                                ps[:, ct, :], w6[:, p, :, co_sl],
                                with_pair(base, delta),
                                start=(p == 0), stop=(p == 8), perf_mode=DR)
                    for ct in range(2):
                        nc.scalar.activation(
                            z7[:, yy, ct, :, :].rearrange(
                                "p x b -> p (x b)"),
                            ps[:, ct, :],
                            AF.Relu, bias=sb6[:, ct, 1:2],
                            scale=sb6[:, ct, 0:1])
                zx = pt.tile([128, 2, 2, 4, B], dt.float32, tag="zx")
                for yy in range(2):
                    nc.vector.tensor_tensor(
                        zx[:, yy, :, :, :], z7[:, yy, :, 0::2, :],
                        z7[:, yy, :, 1::2, :], OP.max)
                zp = pt.tile([128, 2, 4, B], dt.float32, tag="zp")
                nc.vector.tensor_tensor(zp[:], zx[:, 0, :, :, :],
                                        zx[:, 1, :, :, :], OP.max)
                zpf = zp[:].rearrange("p c x b -> p (c x b)")
                nc.vector.tensor_scalar(zpf, zpf, MAGIC, MAGIC + 7.0,
                                        OP.add, OP.min)
                nc.scalar.activation(A7[:, :, yo, :, :], zp[:],
                                     AF.Copy, bias=-MAGIC)

        # ------------- FC1 (4096 -> 512) ----------------------------------
        pa8_cm = tc.tile_pool(name="A8", bufs=1)
        pa8 = pa8_cm.__enter__()
        A8 = pa8.tile([128, 4, B], FP8, tag="A8")
        with (tc.tile_pool(name="f1ps", bufs=4, space="PSUM") as pps,
              tc.tile_pool(name="f1t", bufs=4) as pt):
            for ct in range(4):
                ps = pps.tile([128, B], dt.float32, tag="ps")
                k = 0
                for cig in range(2):
                    for px in range(16):
                        wo = (cig * 16 + px) * 512 + ct * 128
                        nc.tensor.matmul(ps[:], wf1[:, wo:wo + 128],
                                         A7[:, cig, px // 4, px % 4, :],
                                         start=(k == 0), stop=(k == 31))
                        k += 1
                z7 = pt.tile([128, B], dt.float32, tag="z")
                nc.scalar.activation(z7[:], ps[:], AF.Relu,
                                     bias=sbf1[:, ct, 1:2],
                                     scale=sbf1[:, ct, 0:1])
                nc.vector.tensor_scalar(z7[:], z7[:], MAGIC, MAGIC + 7.0,
                                        OP.add, OP.min)
                nc.scalar.activation(A8[:, ct, :], z7[:], AF.Copy,
                                      bias=-MAGIC)

        # ------------- FC2 (512 -> 10), signed output ---------------------
        with (tc.tile_pool(name="f2ps", bufs=1, space="PSUM") as pps,
              tc.tile_pool(name="f2t", bufs=1) as pt):
            ps = pps.tile([10, B], dt.float32, tag="ps")
            for kt in range(4):
                nc.tensor.matmul(ps[:], wf2[:, kt * 10:(kt + 1) * 10],
                                 A8[:, kt, :], start=(kt == 0), stop=(kt == 3))
            z7 = pt.tile([10, B], dt.float32, tag="z")
            nc.vector.tensor_scalar(z7[:], ps[:], sbf2[:, 0:1], sbf2[:, 1:2],
                                    OP.mult, OP.add)
            r = pt.tile([10, B], dt.float32, tag="r")
            nc.vector.tensor_scalar(r[:], z7[:], MAGIC, MAGIC - 7.0,
                                    OP.add, OP.max)  # RNE + lower clamp
            r2 = pt.tile([10, B], dt.float32, tag="r2")
            nc.vector.tensor_scalar(r2[:], r[:], MAGIC + 7.0, MAGIC,
                                    OP.min, OP.subtract)
            fin = pt.tile([10, B], dt.float32, tag="fin")
            nc.vector.tensor_scalar(fin[:], r2[:], 1.0 / 7.0,
                                    None, OP.mult)
            nc.sync.dma_start(outd[:].rearrange("b c -> c b"), fin[:])
        for cm in (pa8_cm, pa7_cm, pa6_cm, pa5_cm, pa4_cm, pa3_cm, fcw_cm,
                   pa2_cm):
            cm.__exit__(None, None, None)
        wp_cm.__exit__(None, None, None)

    nc.compile()
    return nc


# ----------------------------------------------------------------------------
# Entry point
# ----------------------------------------------------------------------------

_NC_CACHE = {}
LAST_RESULTS = None  # BassKernelResults of the most recent run (for test.py)


def kernel(**inputs):
    global LAST_RESULTS
    from concourse.bass_utils import run_bass_kernel_spmd
    if "nc" not in _NC_CACHE:
        _NC_CACHE["nc"] = build_nc()
    nc = _NC_CACHE["nc"]
    in_maps = host_pack(inputs)
    res = run_bass_kernel_spmd(nc, in_maps, list(range(N_CORES)))
    LAST_RESULTS = res
    outs = [res.results[c]["out"] for c in range(N_CORES)]
    return np.concatenate(outs, axis=0).astype(np.float32)


# revision 13
# speedup vs baseline: 3.6945x; 1.0341x over previous
"""Trainium2 Bass kernel for nn_IntegerCifar10Net (quantized VGG-ish CNN).

Data parallel over 8 NeuronCores, B=64 images/core.

v3: engine-balanced quant chain + upfront chunked xcol streaming.

Layer matmul schemes (unchanged from v2 except L6):
  L1 : exact 3-plane bf16 im2col (K=81), co=64; x-halves to PSUM partitions
       0-63 / 64-127; quantized row writes A2 lower+upper halves in one op.
  L2 : "halves" trick - PSUM partitions 0-63 = left 16 output cols,
       64-127 = right 16; block-diagonal weights, 5 DR matmuls per bank.
  L3 : dx-packing (x+1 dup in upper partitions): 3 DR matmuls per bank.
  L4/L5: plain 9 taps -> 5 DR matmuls per bank.
  L6 : 9 DR matmuls per bank - the two odd 9th taps of the two ci-groups
       share one DR pair (cig-pairing) instead of 2 zero-padded pairs.

Quant chain per bank, balanced across Scalar(Act) and Vector(DVE) (the
GpSimd/Pool engine only has slow Q7-ucode elementwise ops on TRN2, and
they also stall DVE via the shared SBUF port - measured 10.6us/op):
  Act : z = relu(psum*scale + bias)                  [PSUM -> SBUF f32]
  DVE : u = min(z + MAGIC, MAGIC+7)  (in-place)      [RNE round + clamp]
  Act/DVE : a = u - MAGIC -> fp8 (Copy activation with immediate bias
  -MAGIC on Act, tensor_scalar on DVE; split to balance engine load)
Max-pool layers run the pairwise maxes on DVE over pre-round z.

xcol is staged fully in SBUF via 8 upfront chunk DMAs (4 rows each)
round-robined over the sync/scalar/gpsimd queues; weights stream behind.
"""

import sys
import numpy as np

sys.path.insert(0, "/opt/trn_rl_repo")

import ml_dtypes

N_CORES = 8
B = 64  # images per core
MAGIC = 12582912.0  # 1.5 * 2^23 : RNE rounding magic for |v| < 2^22
N_PLANES = 3  # bf16 planes for exact L1 input (hi/mid/lo)
K1 = 27 * N_PLANES

# tap pair schedule for 3x3 convs: pairs of taps t=(dy,dx) row-major,
# 10th tap is zero-weight padding with moving delta -B (always in bounds)
TAPS = [(dy, dx) for dy in range(3) for dx in range(3)]


def _pair_deltas(W):
    """Moving-AP element deltas between the two taps of each DR pair."""
    ds = []
    for p in range(4):
        (dya, dxa), (dyb, dxb) = TAPS[2 * p], TAPS[2 * p + 1]
        ds.append(((dyb - dya) * W + (dxb - dxa)) * B)
    ds.append(-B)  # pad pair: (t8, zero-weight tap at x-1)
    return ds


# ----------------------------------------------------------------------------
# Host-side packing
# ----------------------------------------------------------------------------

def _qint(w):
    """round(clip(w,-1,1)*7) as float32 integers, matching jax fp32 chain."""
    w = np.asarray(w, np.float32)
    return np.round(np.clip(w, -1.0, 1.0) * np.float32(7.0)).astype(np.float32)


def _scale_bias(g, b, denom):
    # z7 = conv_int * (7*g/denom) + 7*b, constants in f64 then rounded to f32
    s = (7.0 * np.asarray(g, np.float64) / denom).astype(np.float32)
    bt = (7.0 * np.asarray(b, np.float64)).astype(np.float32)
    return np.ascontiguousarray(np.stack([s, bt], axis=1))  # [co, 2] f32


def _im2col_bf16(x):
    """x [B,3,32,32] f32 -> [K1, 32, 32, B] bf16 (N_PLANES x 27 rows)."""
    Bc = x.shape[0]
    xp = np.zeros((Bc, 3, 34, 34), np.float32)
    xp[:, :, 1:33, 1:33] = x
    planes = np.empty((27, 32, 32, Bc), np.float32)
    k = 0
    for ci in range(3):
        for dy in range(3):
            for dx in range(3):
                planes[k] = np.transpose(xp[:, ci, dy:dy + 32, dx:dx + 32],
                                         (1, 2, 0))
                k += 1
    out = []
    rem = planes
    for _ in range(N_PLANES):
        p = rem.astype(ml_dtypes.bfloat16)
        out.append(p)
        rem = rem - p.astype(np.float32)
    return np.ascontiguousarray(np.concatenate(out, axis=0))


def host_pack(inputs):
    """Build the per-core DRAM input dicts (weights replicated)."""
    f8 = ml_dtypes.float8_e4m3
    wc = {}
    # L1 weights: [64,3,3,3] -> lhsT [27,64], replicated per plane
    t = np.transpose(_qint(inputs["w1"]), (1, 2, 3, 0)).reshape(27, 64)
    wc["w1sb"] = np.ascontiguousarray(
        np.concatenate([t] * N_PLANES, axis=0).astype(ml_dtypes.bfloat16))
    sb = _scale_bias(inputs["g1"], inputs["b1"], 7.0)  # [64, 2]
    wc["sb1"] = np.ascontiguousarray(
        np.concatenate([sb, sb], axis=0).reshape(128, 1, 2))
    # L2: halves block-diagonal [128, 10, 128]
    wq = _qint(inputs["w2"])  # [64co, 64ci, 3, 3]
    w2p = np.zeros((128, 10, 128), np.float32)
    for ti, (dy, dx) in enumerate(TAPS):
        blk = wq[:, :, dy, dx].T  # [ci, co]
        w2p[0:64, ti, 0:64] = blk
        w2p[64:128, ti, 64:128] = blk
    wc["w2sb"] = np.ascontiguousarray(w2p.astype(f8))
    sb = _scale_bias(inputs["g2"], inputs["b2"], 49.0)
    wc["sb2"] = np.ascontiguousarray(
        np.concatenate([sb, sb], axis=0).reshape(128, 1, 2))
    # L3: dx-packed virtual taps [128, 6, 128]
    wq = _qint(inputs["w3"])  # [128co, 64ci, 3, 3]
    w3p = np.zeros((128, 6, 128), np.float32)
    for dy in range(3):
        for oi, o in enumerate((0, 2)):
            v = 2 * dy + oi
            w3p[0:64, v, :] = wq[:, :, dy, o].T
            if o == 0:
                w3p[64:128, v, :] = wq[:, :, dy, 1].T
    wc["w3sb"] = np.ascontiguousarray(w3p.astype(f8))
    wc["sb3"] = np.ascontiguousarray(
        _scale_bias(inputs["g3"], inputs["b3"], 49.0).reshape(128, 1, 2))
    # L4: [128, 10, 128]
    wq = _qint(inputs["w4"])  # [128, 128, 3, 3]
    w4p = np.zeros((128, 10, 128), np.float32)
    for ti, (dy, dx) in enumerate(TAPS):
        w4p[:, ti, :] = wq[:, :, dy, dx].T
    wc["w4sb"] = np.ascontiguousarray(w4p.astype(f8))
    wc["sb4"] = np.ascontiguousarray(
        _scale_bias(inputs["g4"], inputs["b4"], 49.0).reshape(128, 1, 2))
    # L5: [128, 10, 256]
    wq = _qint(inputs["w5"])  # [256, 128, 3, 3]
    w5p = np.zeros((128, 10, 256), np.float32)
    for ti, (dy, dx) in enumerate(TAPS):
        w5p[:, ti, :] = wq[:, :, dy, dx].T
    wc["w5sb"] = np.ascontiguousarray(w5p.astype(f8))
    wc["sb5"] = np.ascontiguousarray(
        _scale_bias(inputs["g5"], inputs["b5"], 49.0).reshape(2, 128, 2)
        .transpose(1, 0, 2))
    # L6: [128, 9, 2, 256] cig-paired: slots 0-3 pair taps (2p,2p+1) of
    # cig0, slots 4-7 the same of cig1, slot 8 pairs (t8@cig0, t8@cig1).
    wq = _qint(inputs["w6"])  # [256, 256, 3, 3]
    w6p = np.zeros((128, 9, 2, 256), np.float32)
    for cig in range(2):
        for p in range(4):
            (dya, dxa), (dyb, dxb) = TAPS[2 * p], TAPS[2 * p + 1]
            w6p[:, 4 * cig + p, 0, :] = wq[:, cig * 128:(cig + 1) * 128,
                                           dya, dxa].T
            w6p[:, 4 * cig + p, 1, :] = wq[:, cig * 128:(cig + 1) * 128,
                                           dyb, dxb].T
    w6p[:, 8, 0, :] = wq[:, 0:128, 2, 2].T
    w6p[:, 8, 1, :] = wq[:, 128:256, 2, 2].T
    wc["w6sb"] = np.ascontiguousarray(w6p.astype(f8))
    wc["sb6"] = np.ascontiguousarray(
        _scale_bias(inputs["g6"], inputs["b6"], 49.0).reshape(2, 128, 2)
        .transpose(1, 0, 2))
    # FC1 [512, 4096]: k=(c,y,x), c=cig*128+p  -> [128, (cig,16,512)]
    t = _qint(inputs["wf1"]).T.reshape(2, 128, 16, 512)
    wc["wf1sb"] = np.ascontiguousarray(
        np.transpose(t, (1, 0, 2, 3)).reshape(128, 2 * 16 * 512).astype(f8))
    wc["sbf1"] = np.ascontiguousarray(_scale_bias(
        inputs["gf1"], inputs["bf1"], 49.0).reshape(4, 128, 2).transpose(
        1, 0, 2))
    # FC2 [10, 512] -> [128, (4,10)]
    t = _qint(inputs["wf2"]).T.reshape(4, 128, 10)
    wc["wf2sb"] = np.ascontiguousarray(
        np.transpose(t, (1, 0, 2)).reshape(128, 40).astype(f8))
    wc["sbf2"] = _scale_bias(inputs["gf2"], inputs["bf2"], 49.0)

    x = np.asarray(inputs["x"], np.float32)
    maps = []
    for c in range(N_CORES):
        m = dict(wc)
        m["xcol"] = _im2col_bf16(x[c * B:(c + 1) * B])
        maps.append(m)
    return maps


# ----------------------------------------------------------------------------
# Bass program
# ----------------------------------------------------------------------------

def build_nc():
    import concourse.bacc as bacc
    import concourse.mybir as mybir
    import concourse.tile as tile

    dt = mybir.dt
    AF = mybir.ActivationFunctionType
    OP = mybir.AluOpType
    FP8 = dt.float8e4
    DR = mybir.MatmulPerfMode.DoubleRow

    nc = bacc.Bacc("TRN2", target_bir_lowering=False, debug=False)

    xcold = nc.dram_tensor("xcol", [K1, 32, 32, B], dt.bfloat16,
                           kind="ExternalInput")
    w1d = nc.dram_tensor("w1sb", [K1, 64], dt.bfloat16, kind="ExternalInput")
    w2d = nc.dram_tensor("w2sb", [128, 10, 128], FP8, kind="ExternalInput")
    w3d = nc.dram_tensor("w3sb", [128, 6, 128], FP8, kind="ExternalInput")
    w4d = nc.dram_tensor("w4sb", [128, 10, 128], FP8, kind="ExternalInput")
    w5d = nc.dram_tensor("w5sb", [128, 10, 256], FP8, kind="ExternalInput")
    w6d = nc.dram_tensor("w6sb", [128, 9, 2, 256], FP8, kind="ExternalInput")
    sbd = {}
    sbshape = {1: [128, 1, 2], 2: [128, 1, 2], 3: [128, 1, 2],
               4: [128, 1, 2], 5: [128, 2, 2], 6: [128, 2, 2]}
    for i in range(1, 7):
        sbd[i] = nc.dram_tensor(f"sb{i}", sbshape[i], dt.float32,
                                kind="ExternalInput")
    wf1d = nc.dram_tensor("wf1sb", [128, 2 * 16 * 512], FP8,
                          kind="ExternalInput")
    sbf1d = nc.dram_tensor("sbf1", [128, 4, 2], dt.float32,
                           kind="ExternalInput")
    wf2d = nc.dram_tensor("wf2sb", [128, 40], FP8, kind="ExternalInput")
    sbf2d = nc.dram_tensor("sbf2", [10, 2], dt.float32, kind="ExternalInput")
    outd = nc.dram_tensor("out", [B, 10], dt.float32, kind="ExternalOutput")

    with tile.TileContext(nc) as tc:
        # ------------- persistent weights (tiles only, DMAs below) --------
        wp_cm = tc.tile_pool(name="weights", bufs=1)
        wp = wp_cm.__enter__()
        w1 = wp.tile([K1, 64], dt.bfloat16, tag="w1")
        wsb = {}
        for i, shape in ((2, [128, 10, 128]), (3, [128, 6, 128]),
                         (4, [128, 10, 128]), (5, [128, 10, 256]),
                         (6, [128, 9, 2, 256])):
            t = wp.tile(shape, FP8, tag=f"w{i}")
            wsb[i] = t
        sbt = {}
        sbt0 = {}
        for i in range(1, 7):
            t0 = wp.tile(sbshape[i], dt.float32, tag=f"s{i}d")
            sbt0[i] = t0
            t = wp.tile(sbshape[i], dt.float32, tag=f"s{i}")
            sbt[i] = t
        wf2 = wp.tile([128, 40], FP8, tag="wf2")
        sbf2t = wp.tile([10, 2], dt.float32, tag="sf2d")
        sbf2 = wp.tile([10, 2], dt.float32, tag="sf2")

        def zero_border(A, Hp, eng=None):
            e = eng or nc.gpsimd
            e.memset(A[:, 0, :, :], 0.0)
            e.memset(A[:, Hp - 1, :, :], 0.0)
            e.memset(A[:, 1:Hp - 1, 0, :], 0.0)
            e.memset(A[:, 1:Hp - 1, Hp - 1, :], 0.0)

        def with_pair(ap, delta):
            APc = type(ap)
            pairs = list(ap.ap)
            return APc(ap.tensor, ap.offset,
                       [pairs[0], [delta, 2]] + list(pairs[1:]))

        def dr_group(ps_ap, wt, co_sl, base_fn, deltas, npairs, extra=None):
            """Accumulate npairs DR matmuls (+ optional extra groups)."""
            for p in range(npairs):
                nc.tensor.matmul(ps_ap, wt[:, 2 * p:2 * p + 2, co_sl],
                                 with_pair(base_fn(p), deltas[p]),
                                 start=(p == 0), stop=(extra is None
                                                       and p == npairs - 1),
                                 perf_mode=DR)
            if extra is not None:
                wt2, base_fn2, deltas2 = extra
                for p in range(npairs):
                    nc.tensor.matmul(ps_ap, wt2[:, 2 * p:2 * p + 2, co_sl],
                                     with_pair(base_fn2(p), deltas2[p]),
                                     start=False, stop=(p == npairs - 1),
                                     perf_mode=DR)

        # ------------- Layer 1: K=81 im2col conv, x-halves in PSUM --------
        # PSUM partitions 0-63 = left 16 output cols, 64-127 = right 16.
        # The fp8 write then fills A2's lower (channels, x) AND upper
        # (x+16 view) halves in one 128-lane op; only two boundary columns
        # (upper col0 = real x15, lower col17 = real x16) need patch DMAs.
        pa2_cm = tc.tile_pool(name="A2", bufs=1)
        pa2 = pa2_cm.__enter__()
        A2 = pa2.tile([128, 34, 18, B], FP8, tag="A2")

        # small weight/scale DMAs first on every queue so nothing waits
        # behind the xcol stream; then xcol as 32 per-row loads, upfront
        nc.sync.dma_start(w1[:], w1d[:])
        nc.sync.dma_start(sbt0[1][:], sbd[1][:])
        nc.scalar.copy(sbt[1][:], sbt0[1][:])
        nc.scalar.dma_start(wsb[2][:], w2d[:])
        nc.scalar.dma_start(wsb[3][:], w3d[:])
        nc.scalar.dma_start(wsb[4][:], w4d[:])
        nc.gpsimd.dma_start(wsb[5][:], w5d[:])
        nc.gpsimd.dma_start(wsb[6][:], w6d[:])
        for i in range(2, 7):
            nc.sync.dma_start(sbt0[i][:], sbd[i][:])
            nc.scalar.copy(sbt[i][:], sbt0[i][:])
        nc.sync.dma_start(wf2[:], wf2d[:])
        nc.sync.dma_start(sbf2t[:], sbf2d[:])
        nc.scalar.copy(sbf2[:], sbf2t[:])
        pxc_cm = tc.tile_pool(name="l1mov", bufs=32)
        pxc = pxc_cm.__enter__()
        chunks = []
        chunk_engs = (nc.sync, nc.scalar, nc.gpsimd)
        for c in range(32):
            ck = pxc.tile([K1, 32, B], dt.bfloat16, tag="chunk")
            chunk_engs[c % 3].dma_start(ck[:], xcold[:, c, :, :])
            chunks.append(ck)
        nc.vector.memset(A2[:, 0, :, :], 0.0)
        nc.vector.memset(A2[:, 33, :, :], 0.0)
        nc.gpsimd.memset(A2[:, 1:33, 0, :], 0.0)
        nc.gpsimd.memset(A2[:, 1:33, 17, :], 0.0)
        def l1_store(q, z7):
            # fp8 store for q's two rows, one on each engine (issued one
            # iteration late so neither engine head-of-line blocks)
            nc.vector.tensor_scalar(
                A2[:, 1 + 2 * q, 1:17, :].rearrange("p x b -> p (x b)"),
                z7[:, 0:2, :].rearrange("p g b -> p (g b)"),
                MAGIC, None, OP.subtract)
            nc.scalar.activation(
                A2[:, 2 + 2 * q, 1:17, :].rearrange("p x b -> p (x b)"),
                z7[:, 2:4, :].rearrange("p g b -> p (g b)"),
                AF.Copy, bias=-MAGIC)

        with (tc.tile_pool(name="l1ps", bufs=2, space="PSUM") as pps,
              tc.tile_pool(name="l1z", bufs=3) as pz):
            prev = None
            for q in range(16):  # y-pairs
                ps = pps.tile([128, 4, 512], dt.float32, tag="ps")
                for yy in range(2):
                    ck = chunks[2 * q + yy]
                    for g in range(2):
                        nc.tensor.matmul(ps[0:64, 2 * yy + g, :], w1[:],
                                         ck[:, g * 8:g * 8 + 8, :],
                                         start=True, stop=True)
                        nc.tensor.matmul(ps[64:128, 2 * yy + g, :], w1[:],
                                         ck[:, 16 + g * 8:
                                            16 + g * 8 + 8, :],
                                         start=True, stop=True)
                z7 = pz.tile([128, 4, 512], dt.float32, tag="z")
                nc.scalar.activation(z7[:], ps[:], AF.Relu,
                                     bias=sbt[1][:, 0, 1:2],
                                     scale=sbt[1][:, 0, 0:1])
                zf = z7[:].rearrange("p a b -> p (a b)")
                nc.vector.tensor_scalar(zf, zf, MAGIC, MAGIC + 7.0,
                                        OP.add, OP.min)
                if prev is not None:
                    l1_store(q - 1, prev)
                prev = z7
            l1_store(15, prev)
        # boundary patch columns (after all xcol chunks: keep queues clean)
        for k in range(4):
            r0 = 1 + 8 * k
            nc.gpsimd.dma_start(A2[64:128, r0:r0 + 8, 0:1, :],
                                A2[0:64, r0:r0 + 8, 16:17, :])
            nc.gpsimd.dma_start(A2[0:64, r0:r0 + 8, 17:18, :],
                                A2[64:128, r0:r0 + 8, 1:2, :])
        pxc_cm.__exit__(None, None, None)  # free the xcol staging space
        fcw_cm = tc.tile_pool(name="fcw", bufs=1)
        fcw = fcw_cm.__enter__()
        wf1 = fcw.tile([128, 2 * 16 * 512], FP8, tag="wf1")
        sbf1t = fcw.tile([128, 4, 2], dt.float32, tag="sf1d")
        sbf1 = fcw.tile([128, 4, 2], dt.float32, tag="sf1")
        nc.scalar.dma_start(sbf1t[:], sbf1d[:])
        nc.scalar.copy(sbf1[:], sbf1t[:])

        # ------------- Layer 2 (64ch 32x32, halves, pool -> 16) -----------
        pa3_cm = tc.tile_pool(name="A3", bufs=1)
        pa3 = pa3_cm.__enter__()
        A3 = pa3.tile([128, 18, 18, B], FP8, tag="A3")
        zero_border(A3, 18)
        d2 = _pair_deltas(18)
        with (tc.tile_pool(name="c2ps", bufs=3, space="PSUM") as pps,
              tc.tile_pool(name="c2z", bufs=2) as pz,
              tc.tile_pool(name="c2t", bufs=2) as pt):
            w2, sb2 = wsb[2], sbt[2]

            def l2_store(yo, zp):
                a3t = pt.tile([128, 2, 4, B], FP8, tag="a3t")
                nc.scalar.activation(
                    a3t[:].rearrange("p a x b -> p (a x b)"),
                    zp[:].rearrange("p a x b -> p (a x b)"),
                    AF.Copy, bias=-MAGIC)
                nc.sync.dma_start(A3[0:64, 1 + yo, 1:9, :],
                                  a3t[0:64].rearrange("p a x b -> p (a x) b"))
                nc.sync.dma_start(A3[0:64, 1 + yo, 9:17, :],
                                  a3t[64:128].rearrange(
                                      "p a x b -> p (a x) b"))

            prev = None
            for yo in range(16):
                z7 = pz.tile([128, 2, 2, 8, B], dt.float32, tag="z")
                for yy in range(2):
                    y = 2 * yo + yy
                    ps = pps.tile([128, 2, 512], dt.float32, tag="ps")
                    for xh in range(2):
                        x0 = 8 * xh

                        def mkbase(p, _y=y, _x0=x0):
                            dy, dx = TAPS[2 * p] if p < 4 else TAPS[8]
                            return A2[:, _y + dy, _x0 + dx:_x0 + dx + 8, :]
                        dr_group(ps[:, xh, :], w2, slice(0, 128), mkbase,
                                 d2, 5)
                    nc.scalar.activation(
                        z7[:, yy, :, :, :].rearrange("p a x b -> p (a x b)")
                        .rearrange("p (a b) -> p a b", b=512),
                        ps[:], AF.Relu, bias=sb2[:, 0, 1:2],
                        scale=sb2[:, 0, 0:1])
                zy = pt.tile([128, 2, 8, B], dt.float32, tag="zy")
                nc.vector.tensor_tensor(zy[:], z7[:, 0, :, :, :],
                                        z7[:, 1, :, :, :], OP.max)
                zp = pt.tile([128, 2, 4, B], dt.float32, tag="zp")
                nc.vector.tensor_tensor(zp[:], zy[:, :, 0::2, :],
                                        zy[:, :, 1::2, :], OP.max)
                zpf = zp[:].rearrange("p a x b -> p (a x b)")
                nc.vector.tensor_scalar(zpf, zpf, MAGIC, MAGIC + 7.0,
                                        OP.add, OP.min)
                if prev is not None:
                    l2_store(yo - 1, prev)
                prev = zp
                # x+1 dup copy for L3 dx-packing, 4 chunks (rows lag 1)
                if yo in (3, 8, 13):
                    r0, r1 = {3: (0, 4), 8: (4, 9), 13: (9, 14)}[yo]
                    nc.sync.dma_start(A3[64:128, r0:r1, 0:17, :],
                                      A3[0:64, r0:r1, 1:18, :])
            l2_store(15, prev)
            nc.sync.dma_start(A3[64:128, 14:18, 0:17, :],
                              A3[0:64, 14:18, 1:18, :])

        nc.gpsimd.dma_start(wf1[:], wf1d[:])

        # ------------- Layer 3 (64 -> 128, 16x16, dx-packed) --------------
        pa4_cm = tc.tile_pool(name="A4", bufs=1)
        pa4 = pa4_cm.__enter__()
        A4 = pa4.tile([128, 18, 18, B], FP8, tag="A4")
        zero_border(A4, 18)
        with (tc.tile_pool(name="c3ps", bufs=3, space="PSUM") as pps,
              tc.tile_pool(name="c3z", bufs=3) as pz):
            w3, sb3 = wsb[3], sbt[3]

            def l3_store(y, z7):
                nc.scalar.activation(
                    A4[:, 1 + y, 1:9, :].rearrange("p x b -> p (x b)"),
                    z7[:, 0, :, :].rearrange("p x b -> p (x b)"),
                    AF.Copy, bias=-MAGIC)
                nc.vector.tensor_scalar(
                    A4[:, 1 + y, 9:17, :].rearrange("p x b -> p (x b)"),
                    z7[:, 1, :, :].rearrange("p x b -> p (x b)"),
                    MAGIC, None, OP.subtract)

            prev = None
            for y in range(16):
                ps = pps.tile([128, 2, 512], dt.float32, tag="ps")
                for xh in range(2):
                    x0 = 8 * xh
                    for dy in range(3):
                        base = A3[:, y + dy, x0:x0 + 8, :]
                        nc.tensor.matmul(
                            ps[:, xh, :], w3[:, 2 * dy:2 * dy + 2, :],
                            with_pair(base, 2 * B), start=(dy == 0),
                            stop=(dy == 2), perf_mode=DR)
                z7 = pz.tile([128, 2, 8, B], dt.float32, tag="z")
                nc.scalar.activation(
                    z7[:].rearrange("p a x b -> p (a x b)")
                    .rearrange("p (a b) -> p a b", b=512),
                    ps[:], AF.Relu, bias=sb3[:, 0, 1:2], scale=sb3[:, 0, 0:1])
                zf = z7[:].rearrange("p a x b -> p (a x b)")
                nc.vector.tensor_scalar(zf, zf, MAGIC, MAGIC + 7.0,
                                        OP.add, OP.min)
                if prev is not None:
                    l3_store(y - 1, prev)
                prev = z7
            l3_store(15, prev)

        # ------------- Layer 4 (128 -> 128, 16x16, pool -> 8) -------------
        pa5_cm = tc.tile_pool(name="A5", bufs=1)
        pa5 = pa5_cm.__enter__()
        A5 = pa5.tile([128, 10, 10, B], FP8, tag="A5")
        zero_border(A5, 10)
        d4 = _pair_deltas(18)
        with (tc.tile_pool(name="c4ps", bufs=3, space="PSUM") as pps,
              tc.tile_pool(name="c4z", bufs=2) as pz,
              tc.tile_pool(name="c4t", bufs=2) as pt):
            w4, sb4 = wsb[4], sbt[4]
            prev = None
            for yo in range(8):
                z7 = pz.tile([128, 2, 2, 8, B], dt.float32, tag="z")
                for yy in range(2):
                    y = 2 * yo + yy
                    ps = pps.tile([128, 2, 512], dt.float32, tag="ps")
                    for xh in range(2):
                        x0 = 8 * xh

                        def mkbase(p, _y=y, _x0=x0):
                            dy, dx = TAPS[2 * p] if p < 4 else TAPS[8]
                            return A4[:, _y + dy, _x0 + dx:_x0 + dx + 8, :]
                        dr_group(ps[:, xh, :], w4, slice(0, 128), mkbase,
                                 d4, 5)
                    nc.scalar.activation(
                        z7[:, yy, :, :, :].rearrange("p a x b -> p (a x b)")
                        .rearrange("p (a b) -> p a b", b=512),
                        ps[:], AF.Relu, bias=sb4[:, 0, 1:2],
                        scale=sb4[:, 0, 0:1])
                zy = pt.tile([128, 2, 8, B], dt.float32, tag="zy")
                nc.vector.tensor_tensor(zy[:], z7[:, 0, :, :, :],
                                        z7[:, 1, :, :, :], OP.max)
                zp = pt.tile([128, 2, 4, B], dt.float32, tag="zp")
                nc.vector.tensor_tensor(zp[:], zy[:, :, 0::2, :],
                                        zy[:, :, 1::2, :], OP.max)
                zpf = zp[:].rearrange("p a x b -> p (a x b)")
                nc.vector.tensor_scalar(zpf, zpf, MAGIC, MAGIC + 7.0,
                                        OP.add, OP.min)
                if prev is not None:
                    nc.scalar.activation(
                        A5[:, yo, 1:9, :].rearrange("p x b -> p (x b)"),
                        prev[:].rearrange("p a x b -> p (a x b)"),
                        AF.Copy, bias=-MAGIC)
                prev = zp
            nc.scalar.activation(
                A5[:, 8, 1:9, :].rearrange("p x b -> p (x b)"),
                prev[:].rearrange("p a x b -> p (a x b)"),
                AF.Copy, bias=-MAGIC)

        # ------------- Layer 5 (128 -> 256, 8x8) --------------------------
        pa6_cm = tc.tile_pool(name="A6", bufs=1)
        pa6 = pa6_cm.__enter__()
        A6 = pa6.tile([128, 2, 10, 10, B], FP8, tag="A6")
        nc.gpsimd.memset(A6[:, :, 0, :, :], 0.0)
        nc.gpsimd.memset(A6[:, :, 9, :, :], 0.0)
        nc.gpsimd.memset(A6[:, :, 1:9, 0, :], 0.0)
        nc.gpsimd.memset(A6[:, :, 1:9, 9, :], 0.0)
        d5 = _pair_deltas(10)
        with (tc.tile_pool(name="c5ps", bufs=3, space="PSUM") as pps,
              tc.tile_pool(name="c5z", bufs=3) as pz):
            w5, sb5 = wsb[5], sbt[5]

            def l5_store(y, z7):
                nc.scalar.activation(
                    A6[:, 0, 1 + y, 1:9, :].rearrange("p x b -> p (x b)"),
                    z7[:, 0, :, :].rearrange("p x b -> p (x b)"),
                    AF.Copy, bias=-MAGIC)
                nc.vector.tensor_scalar(
                    A6[:, 1, 1 + y, 1:9, :].rearrange("p x b -> p (x b)"),
                    z7[:, 1, :, :].rearrange("p x b -> p (x b)"),
                    MAGIC, None, OP.subtract)

            prev = None
            for y in range(8):
                ps = pps.tile([128, 2, 512], dt.float32, tag="ps")
                for ct in range(2):
                    def mkbase(p, _y=y):
                        dy, dx = TAPS[2 * p] if p < 4 else TAPS[8]
                        return A5[:, _y + dy, dx:dx + 8, :]
                    dr_group(ps[:, ct, :], w5,
                             slice(ct * 128, ct * 128 + 128), mkbase, d5, 5)
                z7 = pz.tile([128, 2, 8, B], dt.float32, tag="z")
                for ct in range(2):
                    nc.scalar.activation(
                        z7[:, ct, :, :].rearrange("p x b -> p (x b)"),
                        ps[:, ct, :],
                        AF.Relu, bias=sb5[:, ct, 1:2], scale=sb5[:, ct, 0:1])
                zf = z7[:].rearrange("p c x b -> p (c x b)")
                nc.vector.tensor_scalar(zf, zf, MAGIC, MAGIC + 7.0,
                                        OP.add, OP.min)
                if prev is not None:
                    l5_store(y - 1, prev)
                prev = z7
            l5_store(7, prev)

        # ------------- Layer 6 (256 -> 256, 8x8, pool -> 4) ---------------
        pa7_cm = tc.tile_pool(name="A7", bufs=1)
        pa7 = pa7_cm.__enter__()
        A7 = pa7.tile([128, 2, 4, 4, B], FP8, tag="A7")  # unpadded, feeds FC
        d6 = _pair_deltas(10)
        CIG = 10 * 10 * B  # element offset between the two ci-groups of A6
        with (tc.tile_pool(name="c6ps", bufs=3, space="PSUM") as pps,
              tc.tile_pool(name="c6z", bufs=2) as pz,
              tc.tile_pool(name="c6t", bufs=2) as pt):
            w6, sb6 = wsb[6], sbt[6]
            for yo in range(4):
                z7 = pz.tile([128, 2, 2, 8, B], dt.float32, tag="z")
                for yy in range(2):
                    y = 2 * yo + yy
                    ps = pps.tile([128, 2, 512], dt.float32, tag="ps")
                    for ct in range(2):
                        co_sl = slice(ct * 128, ct * 128 + 128)
                        for p in range(9):
                            if p < 8:
                                cig, pp = p // 4, p % 4
                                dy, dx = TAPS[2 * pp]
                                base = A6[:, cig, y + dy, dx:dx + 8, :]
                                delta = d6[pp]
                            else:
                                base = A6[:, 0, y + 2, 2:2 + 8, :]
                                delta = CIG
                            nc.tensor.matmul(
                                ps[:, ct, :], w6[:, p, :, co_sl],
                                with_pair(base, delta),
                                start=(p == 0), stop=(p == 8), perf_mode=DR)
                    for ct in range(2):
                        nc.scalar.activation(
                            z7[:, yy, ct, :, :].rearrange(
                                "p x b -> p (x b)"),
                            ps[:, ct, :],
                            AF.Relu, bias=sb6[:, ct, 1:2],
                            scale=sb6[:, ct, 0:1])
                zx = pt.tile([128, 2, 2, 4, B], dt.float32, tag="zx")
                for yy in range(2):
                    nc.vector.tensor_tensor(
                        zx[:, yy, :, :, :], z7[:, yy, :, 0::2, :],
                        z7[:, yy, :, 1::2, :], OP.max)
                zp = pt.tile([128, 2, 4, B], dt.float32, tag="zp")
                nc.vector.tensor_tensor(zp[:], zx[:, 0, :, :, :],
                                        zx[:, 1, :, :, :], OP.max)
                zpf = zp[:].rearrange("p c x b -> p (c x b)")
                nc.vector.tensor_scalar(zpf, zpf, MAGIC, MAGIC + 7.0,
                                        OP.add, OP.min)
                nc.scalar.activation(A7[:, :, yo, :, :], zp[:],
                                     AF.Copy, bias=-MAGIC)

        # ------------- FC1 (4096 -> 512) ----------------------------------
        pa8_cm = tc.tile_pool(name="A8", bufs=1)
        pa8 = pa8_cm.__enter__()
        A8 = pa8.tile([128, 4, B], FP8, tag="A8")
        with (tc.tile_pool(name="f1ps", bufs=4, space="PSUM") as pps,
              tc.tile_pool(name="f1t", bufs=4) as pt):
            for ct in range(4):
                ps = pps.tile([128, B], dt.float32, tag="ps")
                k = 0
                for cig in range(2):
                    for px in range(16):
                        wo = (cig * 16 + px) * 512 + ct * 128
                        nc.tensor.matmul(ps[:], wf1[:, wo:wo + 128],
                                         A7[:, cig, px // 4, px % 4, :],
                                         start=(k == 0), stop=(k == 31))
                        k += 1
                z7 = pt.tile([128, B], dt.float32, tag="z")
                nc.scalar.activation(z7[:], ps[:], AF.Relu,
                                     bias=sbf1[:, ct, 1:2],
                                     scale=sbf1[:, ct, 0:1])
                nc.vector.tensor_scalar(z7[:], z7[:], MAGIC, MAGIC + 7.0,
                                        OP.add, OP.min)
                nc.scalar.activation(A8[:, ct, :], z7[:], AF.Copy,
                                      bias=-MAGIC)

        # ------------- FC2 (512 -> 10), signed output ---------------------
        with (tc.tile_pool(name="f2ps", bufs=1, space="PSUM") as pps,
              tc.tile_pool(name="f2t", bufs=1) as pt):
            ps = pps.tile([10, B], dt.float32, tag="ps")
            for kt in range(4):
                nc.tensor.matmul(ps[:], wf2[:, kt * 10:(kt + 1) * 10],
                                 A8[:, kt, :], start=(kt == 0), stop=(kt == 3))
            z7 = pt.tile([10, B], dt.float32, tag="z")
            nc.vector.tensor_scalar(z7[:], ps[:], sbf2[:, 0:1], sbf2[:, 1:2],
                                    OP.mult, OP.add)
            r = pt.tile([10, B], dt.float32, tag="r")
            nc.vector.tensor_scalar(r[:], z7[:], MAGIC, MAGIC - 7.0,
                                    OP.add, OP.max)  # RNE + lower clamp
            r2 = pt.tile([10, B], dt.float32, tag="r2")
            nc.vector.tensor_scalar(r2[:], r[:], MAGIC + 7.0, MAGIC,
                                    OP.min, OP.subtract)
            fin = pt.tile([10, B], dt.float32, tag="fin")
            nc.vector.tensor_scalar(fin[:], r2[:], 1.0 / 7.0,
                                    None, OP.mult)
            nc.sync.dma_start(outd[:].rearrange("b c -> c b"), fin[:])
        for cm in (pa8_cm, pa7_cm, pa6_cm, pa5_cm, pa4_cm, pa3_cm, fcw_cm,
                   pa2_cm):
            cm.__exit__(None, None, None)
        wp_cm.__exit__(None, None, None)

    nc.compile()
    return nc


# ----------------------------------------------------------------------------
# Entry point
# ----------------------------------------------------------------------------

_NC_CACHE = {}
LAST_RESULTS = None  # BassKernelResults of the most recent run (for test.py)


def kernel(**inputs):
    global LAST_RESULTS
    from concourse.bass_utils import run_bass_kernel_spmd
    if "nc" not in _NC_CACHE:
        _NC_CACHE["nc"] = build_nc()
    nc = _NC_CACHE["nc"]
    in_maps = host_pack(inputs)
    res = run_bass_kernel_spmd(nc, in_maps, list(range(N_CORES)))
    LAST_RESULTS = res
    outs = [res.results[c]["out"] for c in range(N_CORES)]
    return np.concatenate(outs, axis=0).astype(np.float32)


# revision 14
# speedup vs baseline: 4.0568x; 1.0981x over previous
"""Trainium2 Bass kernel for nn_IntegerCifar10Net (quantized VGG-ish CNN).

Data parallel over 8 NeuronCores, B=64 images/core.

v3: engine-balanced quant chain + upfront chunked xcol streaming.

Layer matmul schemes (unchanged from v2 except L6):
  L1 : exact 3-plane bf16 im2col (K=81), co=64; x-halves to PSUM partitions
       0-63 / 64-127; quantized row writes A2 lower+upper halves in one op.
  L2 : "halves" trick - PSUM partitions 0-63 = left 16 output cols,
       64-127 = right 16; block-diagonal weights, 5 DR matmuls per bank.
  L3 : dx-packing (x+1 dup in upper partitions): 3 DR matmuls per bank.
  L4/L5: plain 9 taps -> 5 DR matmuls per bank.
  L6 : 9 DR matmuls per bank - the two odd 9th taps of the two ci-groups
       share one DR pair (cig-pairing) instead of 2 zero-padded pairs.

Quant chain per bank, balanced across Scalar(Act) and Vector(DVE) (the
GpSimd/Pool engine only has slow Q7-ucode elementwise ops on TRN2, and
they also stall DVE via the shared SBUF port - measured 10.6us/op):
  Act : z = relu(psum*scale + bias)                  [PSUM -> SBUF f32]
  DVE : u = min(z + MAGIC, MAGIC+7)  (in-place)      [RNE round + clamp]
  Act/DVE : a = u - MAGIC -> fp8 (Copy activation with immediate bias
  -MAGIC on Act, tensor_scalar on DVE; split to balance engine load)
Max-pool layers run the pairwise maxes on DVE over pre-round z.

xcol is staged fully in SBUF via 8 upfront chunk DMAs (4 rows each)
round-robined over the sync/scalar/gpsimd queues; weights stream behind.
"""

import sys
import numpy as np

sys.path.insert(0, "/opt/trn_rl_repo")

import ml_dtypes

N_CORES = 8
B = 64  # images per core
MAGIC = 12582912.0  # 1.5 * 2^23 : RNE rounding magic for |v| < 2^22
N_PLANES = 3  # bf16 planes for exact L1 input (hi/mid/lo)
K1 = 27 * N_PLANES

# tap pair schedule for 3x3 convs: pairs of taps t=(dy,dx) row-major,
# 10th tap is zero-weight padding with moving delta -B (always in bounds)
TAPS = [(dy, dx) for dy in range(3) for dx in range(3)]


def _pair_deltas(W):
    """Moving-AP element deltas between the two taps of each DR pair."""
    ds = []
    for p in range(4):
        (dya, dxa), (dyb, dxb) = TAPS[2 * p], TAPS[2 * p + 1]
        ds.append(((dyb - dya) * W + (dxb - dxa)) * B)
    ds.append(-B)  # pad pair: (t8, zero-weight tap at x-1)
    return ds


# ----------------------------------------------------------------------------
# Host-side packing
# ----------------------------------------------------------------------------

def _qint(w):
    """round(clip(w,-1,1)*7) as float32 integers, matching jax fp32 chain."""
    w = np.asarray(w, np.float32)
    return np.round(np.clip(w, -1.0, 1.0) * np.float32(7.0)).astype(np.float32)


def _scale_bias(g, b, denom):
    # z7 = conv_int * (7*g/denom) + 7*b, constants in f64 then rounded to f32
    s = (7.0 * np.asarray(g, np.float64) / denom).astype(np.float32)
    bt = (7.0 * np.asarray(b, np.float64)).astype(np.float32)
    return np.ascontiguousarray(np.stack([s, bt], axis=1))  # [co, 2] f32


def _im2col_bf16(x):
    """x [B,3,32,32] f32 -> [K1, 32, 32, B] bf16 (N_PLANES x 27 rows)."""
    Bc = x.shape[0]
    xp = np.zeros((Bc, 3, 34, 34), np.float32)
    xp[:, :, 1:33, 1:33] = x
    planes = np.empty((27, 32, 32, Bc), np.float32)
    k = 0
    for ci in range(3):
        for dy in range(3):
            for dx in range(3):
                planes[k] = np.transpose(xp[:, ci, dy:dy + 32, dx:dx + 32],
                                         (1, 2, 0))
                k += 1
    out = []
    rem = planes
    for _ in range(N_PLANES):
        p = rem.astype(ml_dtypes.bfloat16)
        out.append(p)
        rem = rem - p.astype(np.float32)
    return np.ascontiguousarray(np.concatenate(out, axis=0))


def host_pack(inputs):
    """Build the per-core DRAM input dicts (weights replicated)."""
    f8 = ml_dtypes.float8_e4m3
    wc = {}
    # L1 weights: [64,3,3,3] -> lhsT [27,64], replicated per plane
    t = np.transpose(_qint(inputs["w1"]), (1, 2, 3, 0)).reshape(27, 64)
    wc["w1sb"] = np.ascontiguousarray(
        np.concatenate([t] * N_PLANES, axis=0).astype(ml_dtypes.bfloat16))
    sb = _scale_bias(inputs["g1"], inputs["b1"], 7.0)  # [64, 2]
    wc["sb1"] = np.ascontiguousarray(
        np.concatenate([sb, sb], axis=0).reshape(128, 1, 2))
    # L2: halves block-diagonal [128, 10, 128]
    wq = _qint(inputs["w2"])  # [64co, 64ci, 3, 3]
    w2p = np.zeros((128, 10, 128), np.float32)
    for ti, (dy, dx) in enumerate(TAPS):
        blk = wq[:, :, dy, dx].T  # [ci, co]
        w2p[0:64, ti, 0:64] = blk
        w2p[64:128, ti, 64:128] = blk
    wc["w2sb"] = np.ascontiguousarray(w2p.astype(f8))
    sb = _scale_bias(inputs["g2"], inputs["b2"], 49.0)
    wc["sb2"] = np.ascontiguousarray(
        np.concatenate([sb, sb], axis=0).reshape(128, 1, 2))
    # L3: dx-packed virtual taps [128, 6, 128]
    wq = _qint(inputs["w3"])  # [128co, 64ci, 3, 3]
    w3p = np.zeros((128, 6, 128), np.float32)
    for dy in range(3):
        for oi, o in enumerate((0, 2)):
            v = 2 * dy + oi
            w3p[0:64, v, :] = wq[:, :, dy, o].T
            if o == 0:
                w3p[64:128, v, :] = wq[:, :, dy, 1].T
    wc["w3sb"] = np.ascontiguousarray(w3p.astype(f8))
    wc["sb3"] = np.ascontiguousarray(
        _scale_bias(inputs["g3"], inputs["b3"], 49.0).reshape(128, 1, 2))
    # L4: [128, 10, 128]
    wq = _qint(inputs["w4"])  # [128, 128, 3, 3]
    w4p = np.zeros((128, 10, 128), np.float32)
    for ti, (dy, dx) in enumerate(TAPS):
        w4p[:, ti, :] = wq[:, :, dy, dx].T
    wc["w4sb"] = np.ascontiguousarray(w4p.astype(f8))
    wc["sb4"] = np.ascontiguousarray(
        _scale_bias(inputs["g4"], inputs["b4"], 49.0).reshape(128, 1, 2))
    # L5: [128, 10, 256]
    wq = _qint(inputs["w5"])  # [256, 128, 3, 3]
    w5p = np.zeros((128, 10, 256), np.float32)
    for ti, (dy, dx) in enumerate(TAPS):
        w5p[:, ti, :] = wq[:, :, dy, dx].T
    wc["w5sb"] = np.ascontiguousarray(w5p.astype(f8))
    wc["sb5"] = np.ascontiguousarray(
        _scale_bias(inputs["g5"], inputs["b5"], 49.0).reshape(2, 128, 2)
        .transpose(1, 0, 2))
    # L6: [128, 9, 2, 256] cig-paired: slots 0-3 pair taps (2p,2p+1) of
    # cig0, slots 4-7 the same of cig1, slot 8 pairs (t8@cig0, t8@cig1).
    wq = _qint(inputs["w6"])  # [256, 256, 3, 3]
    w6p = np.zeros((128, 9, 2, 256), np.float32)
    for cig in range(2):
        for p in range(4):
            (dya, dxa), (dyb, dxb) = TAPS[2 * p], TAPS[2 * p + 1]
            w6p[:, 4 * cig + p, 0, :] = wq[:, cig * 128:(cig + 1) * 128,
                                           dya, dxa].T
            w6p[:, 4 * cig + p, 1, :] = wq[:, cig * 128:(cig + 1) * 128,
                                           dyb, dxb].T
    w6p[:, 8, 0, :] = wq[:, 0:128, 2, 2].T
    w6p[:, 8, 1, :] = wq[:, 128:256, 2, 2].T
    wc["w6sb"] = np.ascontiguousarray(w6p.astype(f8))
    wc["sb6"] = np.ascontiguousarray(
        _scale_bias(inputs["g6"], inputs["b6"], 49.0).reshape(2, 128, 2)
        .transpose(1, 0, 2))
    # FC1 [512, 4096]: k=(c,y,x), c=cig*128+p  -> [128, (cig,16,512)]
    t = _qint(inputs["wf1"]).T.reshape(2, 128, 16, 512)
    wc["wf1sb"] = np.ascontiguousarray(
        np.transpose(t, (1, 0, 2, 3)).reshape(128, 2 * 16 * 512).astype(f8))
    wc["sbf1"] = np.ascontiguousarray(_scale_bias(
        inputs["gf1"], inputs["bf1"], 49.0).reshape(4, 128, 2).transpose(
        1, 0, 2))
    # FC2 [10, 512] -> [128, (4,10)]
    t = _qint(inputs["wf2"]).T.reshape(4, 128, 10)
    wc["wf2sb"] = np.ascontiguousarray(
        np.transpose(t, (1, 0, 2)).reshape(128, 40).astype(f8))
    wc["sbf2"] = _scale_bias(inputs["gf2"], inputs["bf2"], 49.0)

    x = np.asarray(inputs["x"], np.float32)
    maps = []
    for c in range(N_CORES):
        m = dict(wc)
        m["xcol"] = _im2col_bf16(x[c * B:(c + 1) * B])
        maps.append(m)
    return maps


# ----------------------------------------------------------------------------
# Bass program
# ----------------------------------------------------------------------------

def build_nc():
    import concourse.bacc as bacc
    import concourse.mybir as mybir
    import concourse.tile as tile

    dt = mybir.dt
    AF = mybir.ActivationFunctionType
    OP = mybir.AluOpType
    FP8 = dt.float8e4
    DR = mybir.MatmulPerfMode.DoubleRow

    nc = bacc.Bacc("TRN2", target_bir_lowering=False, debug=False)

    xcold = nc.dram_tensor("xcol", [K1, 32, 32, B], dt.bfloat16,
                           kind="ExternalInput")
    w1d = nc.dram_tensor("w1sb", [K1, 64], dt.bfloat16, kind="ExternalInput")
    w2d = nc.dram_tensor("w2sb", [128, 10, 128], FP8, kind="ExternalInput")
    w3d = nc.dram_tensor("w3sb", [128, 6, 128], FP8, kind="ExternalInput")
    w4d = nc.dram_tensor("w4sb", [128, 10, 128], FP8, kind="ExternalInput")
    w5d = nc.dram_tensor("w5sb", [128, 10, 256], FP8, kind="ExternalInput")
    w6d = nc.dram_tensor("w6sb", [128, 9, 2, 256], FP8, kind="ExternalInput")
    sbd = {}
    sbshape = {1: [128, 1, 2], 2: [128, 1, 2], 3: [128, 1, 2],
               4: [128, 1, 2], 5: [128, 2, 2], 6: [128, 2, 2]}
    for i in range(1, 7):
        sbd[i] = nc.dram_tensor(f"sb{i}", sbshape[i], dt.float32,
                                kind="ExternalInput")
    wf1d = nc.dram_tensor("wf1sb", [128, 2 * 16 * 512], FP8,
                          kind="ExternalInput")
    sbf1d = nc.dram_tensor("sbf1", [128, 4, 2], dt.float32,
                           kind="ExternalInput")
    wf2d = nc.dram_tensor("wf2sb", [128, 40], FP8, kind="ExternalInput")
    sbf2d = nc.dram_tensor("sbf2", [10, 2], dt.float32, kind="ExternalInput")
    outd = nc.dram_tensor("out", [B, 10], dt.float32, kind="ExternalOutput")

    with tile.TileContext(nc) as tc:
        # ------------- persistent weights (tiles only, DMAs below) --------
        wp_cm = tc.tile_pool(name="weights", bufs=1)
        wp = wp_cm.__enter__()
        w1 = wp.tile([K1, 64], dt.bfloat16, tag="w1")
        wsb = {}
        for i, shape in ((2, [128, 10, 128]), (3, [128, 6, 128]),
                         (4, [128, 10, 128]), (5, [128, 10, 256]),
                         (6, [128, 9, 2, 256])):
            t = wp.tile(shape, FP8, tag=f"w{i}")
            wsb[i] = t
        sbt = {}
        sbt0 = {}
        for i in range(1, 7):
            t0 = wp.tile(sbshape[i], dt.float32, tag=f"s{i}d")
            sbt0[i] = t0
            t = wp.tile(sbshape[i], dt.float32, tag=f"s{i}")
            sbt[i] = t
        wf2 = wp.tile([128, 40], FP8, tag="wf2")
        sbf2t = wp.tile([10, 2], dt.float32, tag="sf2d")
        sbf2 = wp.tile([10, 2], dt.float32, tag="sf2")

        def zero_border(A, Hp, eng=None):
            e = eng or nc.gpsimd
            e.memset(A[:, 0, :, :], 0.0)
            e.memset(A[:, Hp - 1, :, :], 0.0)
            e.memset(A[:, 1:Hp - 1, 0, :], 0.0)
            e.memset(A[:, 1:Hp - 1, Hp - 1, :], 0.0)

        def with_pair(ap, delta):
            APc = type(ap)
            pairs = list(ap.ap)
            return APc(ap.tensor, ap.offset,
                       [pairs[0], [delta, 2]] + list(pairs[1:]))

        def dr_group(ps_ap, wt, co_sl, base_fn, deltas, npairs, extra=None):
            """Accumulate npairs DR matmuls (+ optional extra groups)."""
            for p in range(npairs):
                nc.tensor.matmul(ps_ap, wt[:, 2 * p:2 * p + 2, co_sl],
                                 with_pair(base_fn(p), deltas[p]),
                                 start=(p == 0), stop=(extra is None
                                                       and p == npairs - 1),
                                 perf_mode=DR)
            if extra is not None:
                wt2, base_fn2, deltas2 = extra
                for p in range(npairs):
                    nc.tensor.matmul(ps_ap, wt2[:, 2 * p:2 * p + 2, co_sl],
                                     with_pair(base_fn2(p), deltas2[p]),
                                     start=False, stop=(p == npairs - 1),
                                     perf_mode=DR)

        # ------------- Layer 1: K=81 im2col conv, x-halves in PSUM --------
        # PSUM partitions 0-63 = left 16 output cols, 64-127 = right 16.
        # The fp8 write then fills A2's lower (channels, x) AND upper
        # (x+16 view) halves in one 128-lane op; only two boundary columns
        # (upper col0 = real x15, lower col17 = real x16) need patch DMAs.
        pa2_cm = tc.tile_pool(name="A2", bufs=1)
        pa2 = pa2_cm.__enter__()
        A2 = pa2.tile([128, 34, 18, B], FP8, tag="A2")

        # small weight/scale DMAs first on every queue so nothing waits
        # behind the xcol stream; then xcol as 32 per-row loads, upfront
        nc.sync.dma_start(w1[:], w1d[:])
        nc.sync.dma_start(sbt0[1][:], sbd[1][:])
        nc.scalar.copy(sbt[1][:], sbt0[1][:])
        nc.scalar.dma_start(wsb[2][:], w2d[:])
        nc.scalar.dma_start(wsb[3][:], w3d[:])
        nc.scalar.dma_start(wsb[4][:], w4d[:])
        nc.gpsimd.dma_start(wsb[5][:], w5d[:])
        nc.gpsimd.dma_start(wsb[6][:], w6d[:])
        for i in range(2, 7):
            nc.sync.dma_start(sbt0[i][:], sbd[i][:])
            nc.scalar.copy(sbt[i][:], sbt0[i][:])
        nc.sync.dma_start(wf2[:], wf2d[:])
        nc.sync.dma_start(sbf2t[:], sbf2d[:])
        nc.scalar.copy(sbf2[:], sbf2t[:])
        pxc_cm = tc.tile_pool(name="l1mov", bufs=10)
        pxc = pxc_cm.__enter__()
        chunks = {}
        chunk_engs = (nc.sync, nc.scalar, nc.gpsimd)

        def issue_rows(qq):
            # issue the two xcol row DMAs for y-pair qq (just-in-time so no
            # engine's DGE ring ever backs up and blocks its compute queue)
            for yy in range(2):
                r = 2 * qq + yy
                ck = pxc.tile([K1, 32, B], dt.bfloat16, tag="chunk")
                chunk_engs[r % 3].dma_start(ck[:], xcold[:, r, :, :])
                chunks[r] = ck

        LOOK = 3
        for qq in range(LOOK):
            issue_rows(qq)
        nc.vector.memset(A2[:, 0, :, :], 0.0)
        nc.vector.memset(A2[:, 33, :, :], 0.0)
        nc.gpsimd.memset(A2[:, 1:33, 0, :], 0.0)
        nc.gpsimd.memset(A2[:, 1:33, 17, :], 0.0)
        def l1_store(q, z7):
            # fp8 store for q's two rows, one on each engine (issued one
            # iteration late so neither engine head-of-line blocks)
            nc.vector.tensor_scalar(
                A2[:, 1 + 2 * q, 1:17, :].rearrange("p x b -> p (x b)"),
                z7[:, 0:2, :].rearrange("p g b -> p (g b)"),
                MAGIC, None, OP.subtract)
            nc.scalar.activation(
                A2[:, 2 + 2 * q, 1:17, :].rearrange("p x b -> p (x b)"),
                z7[:, 2:4, :].rearrange("p g b -> p (g b)"),
                AF.Copy, bias=-MAGIC)

        with (tc.tile_pool(name="l1ps", bufs=2, space="PSUM") as pps,
              tc.tile_pool(name="l1z", bufs=3) as pz):
            prev = None
            for q in range(16):  # y-pairs
                if q + LOOK < 16:
                    issue_rows(q + LOOK)
                ps = pps.tile([128, 4, 512], dt.float32, tag="ps")
                for yy in range(2):
                    ck = chunks[2 * q + yy]
                    for g in range(2):
                        nc.tensor.matmul(ps[0:64, 2 * yy + g, :], w1[:],
                                         ck[:, g * 8:g * 8 + 8, :],
                                         start=True, stop=True)
                        nc.tensor.matmul(ps[64:128, 2 * yy + g, :], w1[:],
                                         ck[:, 16 + g * 8:
                                            16 + g * 8 + 8, :],
                                         start=True, stop=True)
                z7 = pz.tile([128, 4, 512], dt.float32, tag="z")
                nc.scalar.activation(z7[:], ps[:], AF.Relu,
                                     bias=sbt[1][:, 0, 1:2],
                                     scale=sbt[1][:, 0, 0:1])
                zf = z7[:].rearrange("p a b -> p (a b)")
                nc.vector.tensor_scalar(zf, zf, MAGIC, MAGIC + 7.0,
                                        OP.add, OP.min)
                if prev is not None:
                    l1_store(q - 1, prev)
                prev = z7
            l1_store(15, prev)
        # boundary patch columns (after all xcol chunks: keep queues clean)
        for k in range(4):
            r0 = 1 + 8 * k
            nc.gpsimd.dma_start(A2[64:128, r0:r0 + 8, 0:1, :],
                                A2[0:64, r0:r0 + 8, 16:17, :])
            nc.gpsimd.dma_start(A2[0:64, r0:r0 + 8, 17:18, :],
                                A2[64:128, r0:r0 + 8, 1:2, :])
        pxc_cm.__exit__(None, None, None)  # free the xcol staging space
        fcw_cm = tc.tile_pool(name="fcw", bufs=1)
        fcw = fcw_cm.__enter__()
        wf1 = fcw.tile([128, 2 * 16 * 512], FP8, tag="wf1")
        sbf1t = fcw.tile([128, 4, 2], dt.float32, tag="sf1d")
        sbf1 = fcw.tile([128, 4, 2], dt.float32, tag="sf1")
        nc.scalar.dma_start(sbf1t[:], sbf1d[:])
        nc.scalar.copy(sbf1[:], sbf1t[:])

        # ------------- Layer 2 (64ch 32x32, halves, pool -> 16) -----------
        pa3_cm = tc.tile_pool(name="A3", bufs=1)
        pa3 = pa3_cm.__enter__()
        A3 = pa3.tile([128, 18, 18, B], FP8, tag="A3")
        zero_border(A3, 18)
        d2 = _pair_deltas(18)
        with (tc.tile_pool(name="c2ps", bufs=3, space="PSUM") as pps,
              tc.tile_pool(name="c2z", bufs=2) as pz,
              tc.tile_pool(name="c2t", bufs=2) as pt):
            w2, sb2 = wsb[2], sbt[2]

            def l2_store(yo, zp):
                a3t = pt.tile([128, 2, 4, B], FP8, tag="a3t")
                nc.scalar.activation(
                    a3t[:].rearrange("p a x b -> p (a x b)"),
                    zp[:].rearrange("p a x b -> p (a x b)"),
                    AF.Copy, bias=-MAGIC)
                nc.sync.dma_start(A3[0:64, 1 + yo, 1:9, :],
                                  a3t[0:64].rearrange("p a x b -> p (a x) b"))
                nc.sync.dma_start(A3[0:64, 1 + yo, 9:17, :],
                                  a3t[64:128].rearrange(
                                      "p a x b -> p (a x) b"))

            prev = None
            for yo in range(16):
                z7 = pz.tile([128, 2, 2, 8, B], dt.float32, tag="z")
                for yy in range(2):
                    y = 2 * yo + yy
                    ps = pps.tile([128, 2, 512], dt.float32, tag="ps")
                    for xh in range(2):
                        x0 = 8 * xh

                        def mkbase(p, _y=y, _x0=x0):
                            dy, dx = TAPS[2 * p] if p < 4 else TAPS[8]
                            return A2[:, _y + dy, _x0 + dx:_x0 + dx + 8, :]
                        dr_group(ps[:, xh, :], w2, slice(0, 128), mkbase,
                                 d2, 5)
                    nc.scalar.activation(
                        z7[:, yy, :, :, :].rearrange("p a x b -> p (a x b)")
                        .rearrange("p (a b) -> p a b", b=512),
                        ps[:], AF.Relu, bias=sb2[:, 0, 1:2],
                        scale=sb2[:, 0, 0:1])
                zy = pt.tile([128, 2, 8, B], dt.float32, tag="zy")
                nc.vector.tensor_tensor(zy[:], z7[:, 0, :, :, :],
                                        z7[:, 1, :, :, :], OP.max)
                zp = pt.tile([128, 2, 4, B], dt.float32, tag="zp")
                nc.vector.tensor_tensor(zp[:], zy[:, :, 0::2, :],
                                        zy[:, :, 1::2, :], OP.max)
                zpf = zp[:].rearrange("p a x b -> p (a x b)")
                nc.vector.tensor_scalar(zpf, zpf, MAGIC, MAGIC + 7.0,
                                        OP.add, OP.min)
                if prev is not None:
                    l2_store(yo - 1, prev)
                prev = zp
                # x+1 dup copy for L3 dx-packing, 4 chunks (rows lag 1)
                if yo in (3, 8, 13):
                    r0, r1 = {3: (0, 4), 8: (4, 9), 13: (9, 14)}[yo]
                    nc.sync.dma_start(A3[64:128, r0:r1, 0:17, :],
                                      A3[0:64, r0:r1, 1:18, :])
            l2_store(15, prev)
            nc.sync.dma_start(A3[64:128, 14:18, 0:17, :],
                              A3[0:64, 14:18, 1:18, :])

        nc.gpsimd.dma_start(wf1[:], wf1d[:])

        # ------------- Layer 3 (64 -> 128, 16x16, dx-packed) --------------
        pa4_cm = tc.tile_pool(name="A4", bufs=1)
        pa4 = pa4_cm.__enter__()
        A4 = pa4.tile([128, 18, 18, B], FP8, tag="A4")
        zero_border(A4, 18)
        with (tc.tile_pool(name="c3ps", bufs=3, space="PSUM") as pps,
              tc.tile_pool(name="c3z", bufs=3) as pz):
            w3, sb3 = wsb[3], sbt[3]

            def l3_store(y, z7):
                nc.scalar.activation(
                    A4[:, 1 + y, 1:9, :].rearrange("p x b -> p (x b)"),
                    z7[:, 0, :, :].rearrange("p x b -> p (x b)"),
                    AF.Copy, bias=-MAGIC)
                nc.vector.tensor_scalar(
                    A4[:, 1 + y, 9:17, :].rearrange("p x b -> p (x b)"),
                    z7[:, 1, :, :].rearrange("p x b -> p (x b)"),
                    MAGIC, None, OP.subtract)

            prev = None
            for y in range(16):
                ps = pps.tile([128, 2, 512], dt.float32, tag="ps")
                for xh in range(2):
                    x0 = 8 * xh
                    for dy in range(3):
                        base = A3[:, y + dy, x0:x0 + 8, :]
                        nc.tensor.matmul(
                            ps[:, xh, :], w3[:, 2 * dy:2 * dy + 2, :],
                            with_pair(base, 2 * B), start=(dy == 0),
                            stop=(dy == 2), perf_mode=DR)
                z7 = pz.tile([128, 2, 8, B], dt.float32, tag="z")
                nc.scalar.activation(
                    z7[:].rearrange("p a x b -> p (a x b)")
                    .rearrange("p (a b) -> p a b", b=512),
                    ps[:], AF.Relu, bias=sb3[:, 0, 1:2], scale=sb3[:, 0, 0:1])
                zf = z7[:].rearrange("p a x b -> p (a x b)")
                nc.vector.tensor_scalar(zf, zf, MAGIC, MAGIC + 7.0,
                                        OP.add, OP.min)
                if prev is not None:
                    l3_store(y - 1, prev)
                prev = z7
            l3_store(15, prev)

        # ------------- Layer 4 (128 -> 128, 16x16, pool -> 8) -------------
        pa5_cm = tc.tile_pool(name="A5", bufs=1)
        pa5 = pa5_cm.__enter__()
        A5 = pa5.tile([128, 10, 10, B], FP8, tag="A5")
        zero_border(A5, 10)
        d4 = _pair_deltas(18)
        with (tc.tile_pool(name="c4ps", bufs=3, space="PSUM") as pps,
              tc.tile_pool(name="c4z", bufs=2) as pz,
              tc.tile_pool(name="c4t", bufs=2) as pt):
            w4, sb4 = wsb[4], sbt[4]
            prev = None
            for yo in range(8):
                z7 = pz.tile([128, 2, 2, 8, B], dt.float32, tag="z")
                for yy in range(2):
                    y = 2 * yo + yy
                    ps = pps.tile([128, 2, 512], dt.float32, tag="ps")
                    for xh in range(2):
                        x0 = 8 * xh

                        def mkbase(p, _y=y, _x0=x0):
                            dy, dx = TAPS[2 * p] if p < 4 else TAPS[8]
                            return A4[:, _y + dy, _x0 + dx:_x0 + dx + 8, :]
                        dr_group(ps[:, xh, :], w4, slice(0, 128), mkbase,
                                 d4, 5)
                    nc.scalar.activation(
                        z7[:, yy, :, :, :].rearrange("p a x b -> p (a x b)")
                        .rearrange("p (a b) -> p a b", b=512),
                        ps[:], AF.Relu, bias=sb4[:, 0, 1:2],
                        scale=sb4[:, 0, 0:1])
                zy = pt.tile([128, 2, 8, B], dt.float32, tag="zy")
                nc.vector.tensor_tensor(zy[:], z7[:, 0, :, :, :],
                                        z7[:, 1, :, :, :], OP.max)
                zp = pt.tile([128, 2, 4, B], dt.float32, tag="zp")
                nc.vector.tensor_tensor(zp[:], zy[:, :, 0::2, :],
                                        zy[:, :, 1::2, :], OP.max)
                zpf = zp[:].rearrange("p a x b -> p (a x b)")
                nc.vector.tensor_scalar(zpf, zpf, MAGIC, MAGIC + 7.0,
                                        OP.add, OP.min)
                if prev is not None:
                    nc.scalar.activation(
                        A5[:, yo, 1:9, :].rearrange("p x b -> p (x b)"),
                        prev[:].rearrange("p a x b -> p (a x b)"),
                        AF.Copy, bias=-MAGIC)
                prev = zp
            nc.scalar.activation(
                A5[:, 8, 1:9, :].rearrange("p x b -> p (x b)"),
                prev[:].rearrange("p a x b -> p (a x b)"),
                AF.Copy, bias=-MAGIC)

        # ------------- Layer 5 (128 -> 256, 8x8) --------------------------
        pa6_cm = tc.tile_pool(name="A6", bufs=1)
        pa6 = pa6_cm.__enter__()
        A6 = pa6.tile([128, 2, 10, 10, B], FP8, tag="A6")
        nc.gpsimd.memset(A6[:, :, 0, :, :], 0.0)
        nc.gpsimd.memset(A6[:, :, 9, :, :], 0.0)
        nc.gpsimd.memset(A6[:, :, 1:9, 0, :], 0.0)
        nc.gpsimd.memset(A6[:, :, 1:9, 9, :], 0.0)
        d5 = _pair_deltas(10)
        with (tc.tile_pool(name="c5ps", bufs=3, space="PSUM") as pps,
              tc.tile_pool(name="c5z", bufs=3) as pz):
            w5, sb5 = wsb[5], sbt[5]

            def l5_store(y, z7):
                nc.scalar.activation(
                    A6[:, 0, 1 + y, 1:9, :].rearrange("p x b -> p (x b)"),
                    z7[:, 0, :, :].rearrange("p x b -> p (x b)"),
                    AF.Copy, bias=-MAGIC)
                nc.vector.tensor_scalar(
                    A6[:, 1, 1 + y, 1:9, :].rearrange("p x b -> p (x b)"),
                    z7[:, 1, :, :].rearrange("p x b -> p (x b)"),
                    MAGIC, None, OP.subtract)

            prev = None
            for y in range(8):
                ps = pps.tile([128, 2, 512], dt.float32, tag="ps")
                for ct in range(2):
                    def mkbase(p, _y=y):
                        dy, dx = TAPS[2 * p] if p < 4 else TAPS[8]
                        return A5[:, _y + dy, dx:dx + 8, :]
                    dr_group(ps[:, ct, :], w5,
                             slice(ct * 128, ct * 128 + 128), mkbase, d5, 5)
                z7 = pz.tile([128, 2, 8, B], dt.float32, tag="z")
                for ct in range(2):
                    nc.scalar.activation(
                        z7[:, ct, :, :].rearrange("p x b -> p (x b)"),
                        ps[:, ct, :],
                        AF.Relu, bias=sb5[:, ct, 1:2], scale=sb5[:, ct, 0:1])
                zf = z7[:].rearrange("p c x b -> p (c x b)")
                nc.vector.tensor_scalar(zf, zf, MAGIC, MAGIC + 7.0,
                                        OP.add, OP.min)
                if prev is not None:
                    l5_store(y - 1, prev)
                prev = z7
            l5_store(7, prev)

        # ------------- Layer 6 (256 -> 256, 8x8, pool -> 4) ---------------
        pa7_cm = tc.tile_pool(name="A7", bufs=1)
        pa7 = pa7_cm.__enter__()
        A7 = pa7.tile([128, 2, 4, 4, B], FP8, tag="A7")  # unpadded, feeds FC
        d6 = _pair_deltas(10)
        CIG = 10 * 10 * B  # element offset between the two ci-groups of A6
        with (tc.tile_pool(name="c6ps", bufs=3, space="PSUM") as pps,
              tc.tile_pool(name="c6z", bufs=2) as pz,
              tc.tile_pool(name="c6t", bufs=2) as pt):
            w6, sb6 = wsb[6], sbt[6]
            for yo in range(4):
                z7 = pz.tile([128, 2, 2, 8, B], dt.float32, tag="z")
                for yy in range(2):
                    y = 2 * yo + yy
                    ps = pps.tile([128, 2, 512], dt.float32, tag="ps")
                    for ct in range(2):
                        co_sl = slice(ct * 128, ct * 128 + 128)
                        for p in range(9):
                            if p < 8:
                                cig, pp = p // 4, p % 4
                                dy, dx = TAPS[2 * pp]
                                base = A6[:, cig, y + dy, dx:dx + 8, :]
                                delta = d6[pp]
                            else:
                                base = A6[:, 0, y + 2, 2:2 + 8, :]
                                delta = CIG
                            nc.tensor.matmul(
                                ps[:, ct, :], w6[:, p, :, co_sl],
                                with_pair(base, delta),
                                start=(p == 0), stop=(p == 8), perf_mode=DR)
                    for ct in range(2):
                        nc.scalar.activation(
                            z7[:, yy, ct, :, :].rearrange(
                                "p x b -> p (x b)"),
                            ps[:, ct, :],
                            AF.Relu, bias=sb6[:, ct, 1:2],
                            scale=sb6[:, ct, 0:1])
                zx = pt.tile([128, 2, 2, 4, B], dt.float32, tag="zx")
                for yy in range(2):
                    nc.vector.tensor_tensor(
                        zx[:, yy, :, :, :], z7[:, yy, :, 0::2, :],
                        z7[:, yy, :, 1::2, :], OP.max)
                zp = pt.tile([128, 2, 4, B], dt.float32, tag="zp")
                nc.vector.tensor_tensor(zp[:], zx[:, 0, :, :, :],
                                        zx[:, 1, :, :, :], OP.max)
                zpf = zp[:].rearrange("p c x b -> p (c x b)")
                nc.vector.tensor_scalar(zpf, zpf, MAGIC, MAGIC + 7.0,
                                        OP.add, OP.min)
                nc.scalar.activation(A7[:, :, yo, :, :], zp[:],
                                     AF.Copy, bias=-MAGIC)

        # ------------- FC1 (4096 -> 512) ----------------------------------
        pa8_cm = tc.tile_pool(name="A8", bufs=1)
        pa8 = pa8_cm.__enter__()
        A8 = pa8.tile([128, 4, B], FP8, tag="A8")
        with (tc.tile_pool(name="f1ps", bufs=4, space="PSUM") as pps,
              tc.tile_pool(name="f1t", bufs=4) as pt):
            for ct in range(4):
                ps = pps.tile([128, B], dt.float32, tag="ps")
                k = 0
                for cig in range(2):
                    for px in range(16):
                        wo = (cig * 16 + px) * 512 + ct * 128
                        nc.tensor.matmul(ps[:], wf1[:, wo:wo + 128],
                                         A7[:, cig, px // 4, px % 4, :],
                                         start=(k == 0), stop=(k == 31))
                        k += 1
                z7 = pt.tile([128, B], dt.float32, tag="z")
                nc.scalar.activation(z7[:], ps[:], AF.Relu,
                                     bias=sbf1[:, ct, 1:2],
                                     scale=sbf1[:, ct, 0:1])
                nc.vector.tensor_scalar(z7[:], z7[:], MAGIC, MAGIC + 7.0,
                                        OP.add, OP.min)
                nc.scalar.activation(A8[:, ct, :], z7[:], AF.Copy,
                                      bias=-MAGIC)

        # ------------- FC2 (512 -> 10), signed output ---------------------
        with (tc.tile_pool(name="f2ps", bufs=1, space="PSUM") as pps,
              tc.tile_pool(name="f2t", bufs=1) as pt):
            ps = pps.tile([10, B], dt.float32, tag="ps")
            for kt in range(4):
                nc.tensor.matmul(ps[:], wf2[:, kt * 10:(kt + 1) * 10],
                                 A8[:, kt, :], start=(kt == 0), stop=(kt == 3))
            z7 = pt.tile([10, B], dt.float32, tag="z")
            nc.vector.tensor_scalar(z7[:], ps[:], sbf2[:, 0:1], sbf2[:, 1:2],
                                    OP.mult, OP.add)
            r = pt.tile([10, B], dt.float32, tag="r")
            nc.vector.tensor_scalar(r[:], z7[:], MAGIC, MAGIC - 7.0,
                                    OP.add, OP.max)  # RNE + lower clamp
            r2 = pt.tile([10, B], dt.float32, tag="r2")
            nc.vector.tensor_scalar(r2[:], r[:], MAGIC + 7.0, MAGIC,
                                    OP.min, OP.subtract)
            fin = pt.tile([10, B], dt.float32, tag="fin")
            nc.vector.tensor_scalar(fin[:], r2[:], 1.0 / 7.0,
                                    None, OP.mult)
            nc.sync.dma_start(outd[:].rearrange("b c -> c b"), fin[:])
        for cm in (pa8_cm, pa7_cm, pa6_cm, pa5_cm, pa4_cm, pa3_cm, fcw_cm,
                   pa2_cm):
            cm.__exit__(None, None, None)
        wp_cm.__exit__(None, None, None)

    nc.compile()
    return nc


# ----------------------------------------------------------------------------
# Entry point
# ----------------------------------------------------------------------------

_NC_CACHE = {}
LAST_RESULTS = None  # BassKernelResults of the most recent run (for test.py)


def kernel(**inputs):
    global LAST_RESULTS
    from concourse.bass_utils import run_bass_kernel_spmd
    if "nc" not in _NC_CACHE:
        _NC_CACHE["nc"] = build_nc()
    nc = _NC_CACHE["nc"]
    in_maps = host_pack(inputs)
    res = run_bass_kernel_spmd(nc, in_maps, list(range(N_CORES)))
    LAST_RESULTS = res
    outs = [res.results[c]["out"] for c in range(N_CORES)]
    return np.concatenate(outs, axis=0).astype(np.float32)
